# revision 1
# baseline (speedup 1.0000x reference)
"""Trainium2 Bass kernel for nn_ExchangeBlock (gnn_message_passing).

Data-parallel over edges: each of the 8 cores processes E/8 = 16384 edges,
node features and weights replicated.  Per 512-edge tile:
  - node features gathered FEATURE-MAJOR via transposing dma_gather (fp16,
    no PE transposes); pos/cell rows via classic indirect DMA
  - radial: tvec, dist (DVE Newton rsqrt), Bessel embedding (range-reduced
    Sin on ACT); embedding transposed to feature-major with an SBUF-source
    transposing dma_gather
  - fp16 matmuls (free=512 -> 1 cycle/row) for the distance-filter MLP, the
    symmetrized tensor product and the mix MLP; LayerNorm stats via
    ones-matmul cross-partition reductions; products and stats math on DVE
All activations/weights fp16 (quantization ~1e-3), radial + LN-stat math
f32.  PSUM: 2 banks replication A, 2 banks B/output-row, 2 accumulation
rotation, 2 stats; the LN broadcast rows reuse the replication banks.
"""
import os
import sys

sys.path.insert(0, "/opt/trn_rl_repo")

import math
import numpy as np

L0, L1, L2 = 32, 16, 8
NS, NB = 512, 256
CUT = 7.0
N, E, G = 16384, 131072, 16
FEAT = L0 + 3 * L1 + 5 * L2  # 120
NCORES = 8
EC = E // NCORES  # edges per core
BLK = 128
ET = 512  # edges per tile (= one PSUM bank of fp32)
NBLK = ET // BLK
FAN = math.sqrt(float(L0 * L0 + L1 * L1 + L2 * L2))
EMBC = math.sqrt(2.0 / CUT)
MAGIC = 0x5F3759DF

_cache = {}


def _build(mode, ntiles, reps=1, stage=99):
    """Build the Bass program (shared by all cores, SPMD)."""
    import concourse.bacc as bacc
    import concourse.bass as bass
    import concourse.mybir as mybir
    import concourse.tile as tile

    f32 = mybir.dt.float32
    f32r = mybir.dt.float32r
    f16 = mybir.dt.float16
    i32 = mybir.dt.int32
    i16 = mybir.dt.int16
    AF = mybir.ActivationFunctionType
    OP = mybir.AluOpType
    AX = mybir.AxisListType

    nc = bacc.Bacc(None)

    # ---------------- DRAM tensors ----------------
    nodesF = nc.dram_tensor("nodesF", [N, BLK], f16, kind="ExternalInput")
    posC = nc.dram_tensor("posC", [N, 16], f32, kind="ExternalInput")
    g16s = nc.dram_tensor("g16s", [ntiles, BLK, ET // 16], i16, kind="ExternalInput")
    g16d = nc.dram_tensor("g16d", [ntiles, BLK, ET // 16], i16, kind="ExternalInput")
    pcidx = nc.dram_tensor("pcidx", [ntiles, BLK, 8], i32, kind="ExternalInput")
    shiftd = nc.dram_tensor("shiftd", [ntiles, BLK, NBLK, 3], f32, kind="ExternalInput")
    identd = nc.dram_tensor("identd", [BLK, BLK], f16, kind="ExternalInput")

    w0p = nc.dram_tensor("w0p", [BLK, 8, NS], f16, kind="ExternalInput")
    w1p = nc.dram_tensor("w1p", [BLK, 2, NS], f16, kind="ExternalInput")
    w2p = nc.dram_tensor("w2p", [64, NS], f16, kind="ExternalInput")
    dfw1p = nc.dram_tensor("dfw1p", [BLK, 2, 1024], f16, kind="ExternalInput")
    dfw2p = nc.dram_tensor("dfw2p", [BLK, 8, NS], f16, kind="ExternalInput")
    miw1p = nc.dram_tensor("miw1p", [BLK, 4, 1024], f16, kind="ExternalInput")
    miw2p = nc.dram_tensor("miw2p", [BLK, 8, 1024], f16, kind="ExternalInput")
    mowp = nc.dram_tensor("mowp", [BLK, 8], f16, kind="ExternalInput")
    s0d = nc.dram_tensor("s0d", [BLK, 8 * BLK], f16, kind="ExternalInput")
    t0d = nc.dram_tensor("t0d", [BLK, BLK], f16, kind="ExternalInput")
    s1d = nc.dram_tensor("s1d", [BLK, 6 * BLK], f16, kind="ExternalInput")
    t1d = nc.dram_tensor("t1d", [BLK, 3 * BLK], f16, kind="ExternalInput")
    s2d = nc.dram_tensor("s2d", [BLK, 5 * 64], f16, kind="ExternalInput")
    t2d = nc.dram_tensor("t2d", [BLK, 5 * 64], f16, kind="ExternalInput")
    bdf1 = nc.dram_tensor("bdf1", [BLK, 8], f32, kind="ExternalInput")
    bdf2 = nc.dram_tensor("bdf2", [BLK, 4], f32, kind="ExternalInput")
    bmi1 = nc.dram_tensor("bmi1", [BLK, 8], f32, kind="ExternalInput")
    bmi2 = nc.dram_tensor("bmi2", [BLK, 8], f32, kind="ExternalInput")
    bmo = nc.dram_tensor("bmo", [1, 1], f32, kind="ExternalInput")
    onesrd = nc.dram_tensor("onesrd", [1, BLK], f32, kind="ExternalInput")
    cnd = nc.dram_tensor("cnd", [BLK, NB], f32, kind="ExternalInput")

    outd = nc.dram_tensor("out", [ntiles, 1, ET], f32, kind="ExternalOutput")

    TWO_PI = 2.0 * math.pi
    sin_bias = -math.pi if mode == "sim" else 0.0
    dscale_c = -EMBC if mode == "sim" else EMBC

    with tile.TileContext(nc) as tc:
        with (
            tc.tile_pool(name="const", bufs=1) as cp,
            tc.tile_pool(name="gat", bufs=3) as gp,
            tc.tile_pool(name="rad", bufs=3) as rp,
            tc.tile_pool(name="emb", bufs=2) as ep,
            tc.tile_pool(name="trx", bufs=3) as xp,
            tc.tile_pool(name="tpp", bufs=2) as tp,
            tc.tile_pool(name="mid", bufs=2) as mp,
            tc.tile_pool(name="row", bufs=1) as wp,
            tc.tile_pool(name="h1s", bufs=2) as hp1,
            tc.tile_pool(name="hs", bufs=2) as hp,
            tc.tile_pool(name="sml", bufs=2) as sp,
            tc.tile_pool(name="psA", bufs=2, space="PSUM") as pgA,
            tc.tile_pool(name="psB", bufs=1, space="PSUM") as pgB,
            tc.tile_pool(name="pac", bufs=3, space="PSUM") as pacc,
            tc.tile_pool(name="pst", bufs=2, space="PSUM") as pst,
        ):
            # ---------------- constants ----------------
            w0_t = cp.tile([BLK, 8, NS], f16)
            nc.sync.dma_start(w0_t[:], w0p[:])
            w1_t = cp.tile([BLK, 2, NS], f16)
            nc.sync.dma_start(w1_t[:], w1p[:])
            w2_t = cp.tile([64, NS], f16)
            nc.sync.dma_start(w2_t[:], w2p[:])
            dfw1_t = cp.tile([BLK, 2, 1024], f16)
            nc.sync.dma_start(dfw1_t[:], dfw1p[:])
            dfw2_t = cp.tile([BLK, 8, NS], f16)
            nc.sync.dma_start(dfw2_t[:], dfw2p[:])
            miw1_t = cp.tile([BLK, 4, 1024], f16)
            nc.sync.dma_start(miw1_t[:], miw1p[:])
            miw2_t = cp.tile([BLK, 8, 1024], f16)
            nc.sync.dma_start(miw2_t[:], miw2p[:])
            mow_t = cp.tile([BLK, 8], f16)
            nc.sync.dma_start(mow_t[:], mowp[:])
            s0_t = cp.tile([BLK, 8 * BLK], f16)
            nc.sync.dma_start(s0_t[:], s0d[:])
            t0_t = cp.tile([BLK, BLK], f16)
            nc.sync.dma_start(t0_t[:], t0d[:])
            s1_t = cp.tile([BLK, 6 * BLK], f16)
            nc.sync.dma_start(s1_t[:], s1d[:])
            t1_t = cp.tile([BLK, 3 * BLK], f16)
            nc.sync.dma_start(t1_t[:], t1d[:])
            s2_t = cp.tile([BLK, 5 * 64], f16)
            nc.sync.dma_start(s2_t[:], s2d[:])
            t2_t = cp.tile([BLK, 5 * 64], f16)
            nc.sync.dma_start(t2_t[:], t2d[:])
            bdf1_t = cp.tile([BLK, 8], f32)
            nc.sync.dma_start(bdf1_t[:], bdf1[:])
            bdf2_t = cp.tile([BLK, 4], f32)
            nc.sync.dma_start(bdf2_t[:], bdf2[:])
            bmi1_t = cp.tile([BLK, 8], f32)
            nc.sync.dma_start(bmi1_t[:], bmi1[:])
            bmi2_t = cp.tile([BLK, 8], f32)
            nc.sync.dma_start(bmi2_t[:], bmi2[:])
            bmo_t = cp.tile([1, 1], f32)
            nc.sync.dma_start(bmo_t[:], bmo[:])
            onesr_t = cp.tile([BLK, BLK], f32)
            nc.gpsimd.memset(onesr_t[:], 1.0)
            cn_t = cp.tile([BLK, NB], f32)
            nc.sync.dma_start(cn_t[:], cnd[:])
            id_t = cp.tile([BLK, BLK], f16)
            nc.sync.dma_start(id_t[:], identd[:])
            onesc_t = cp.tile([BLK, 1], f16)
            nc.gpsimd.memset(onesc_t[:], 1.0)
            magic_t = cp.tile([BLK, NBLK], i32)
            nc.gpsimd.memset(magic_t[:], MAGIC)
            magicr_t = cp.tile([1, NS], i32)
            nc.gpsimd.memset(magicr_t[:], MAGIC)
            sinb_t = cp.tile([BLK, 1], f32)
            nc.gpsimd.memset(sinb_t[:], sin_bias)

            def silu_to(dst, ps, bias_ap):
                if mode == "sim":
                    sg = sp.tile([BLK, ET], f16, tag="sg")
                    nc.scalar.activation(sg[:], ps, AF.Sigmoid, bias=bias_ap, scale=1.0)
                    pre = sp.tile([BLK, ET], f16, tag="pre")
                    nc.vector.tensor_scalar(out=pre[:], in0=ps, scalar1=bias_ap,
                                            scalar2=None, op0=OP.add)
                    nc.vector.tensor_tensor(out=dst, in0=sg[:], in1=pre[:], op=OP.mult)
                else:
                    nc.scalar.activation(dst, ps, AF.Silu, bias=bias_ap, scale=1.0)

            def front(t):
                # ============ index DMAs ============
                if stage < 0:
                    return None
                gs16 = gp.tile([BLK, ET // 16], i16, tag="gs16")
                gd16 = gp.tile([BLK, ET // 16], i16, tag="gd16")
                nc.sync.dma_start(gs16[:], g16s[t])
                nc.sync.dma_start(gd16[:], g16d[t])
                pci = gp.tile([BLK, 8], i32, tag="pci")
                nc.sync.dma_start(pci[:], pcidx[t])
                shf = gp.tile([BLK, NBLK, 3], f32, tag="shf")
                nc.sync.dma_start(shf[:], shiftd[t])

                # ============ gathers ============
                xsT = xp.tile([BLK, 1, ET], f16, tag="xsT")
                nc.gpsimd.dma_gather(
                    out_ap=xsT[:], in_ap=nodesF[:], idxs_ap=gs16[:],
                    num_idxs=ET, num_idxs_reg=ET, elem_size=BLK, transpose=True)
                xdT = xp.tile([BLK, 1, ET], f16, tag="xdT")
                nc.gpsimd.dma_gather(
                    out_ap=xdT[:], in_ap=nodesF[:], idxs_ap=gd16[:],
                    num_idxs=ET, num_idxs_reg=ET, elem_size=BLK, transpose=True)
                gsc = gp.tile([BLK, NBLK, 16], f32, tag="gsc")
                gdc = gp.tile([BLK, NBLK, 16], f32, tag="gdc")
                for b in range(NBLK):
                    nc.gpsimd.indirect_dma_start(
                        out=gsc[:, b, :], out_offset=None, in_=posC[:],
                        in_offset=bass.IndirectOffsetOnAxis(ap=pci[:, b:b + 1], axis=0))
                    nc.gpsimd.indirect_dma_start(
                        out=gdc[:, b, :], out_offset=None, in_=posC[:],
                        in_offset=bass.IndirectOffsetOnAxis(ap=pci[:, 4 + b:5 + b], axis=0))

                if stage < 1:
                    ot0 = sp.tile([1, ET], f32, tag="ot")
                    nc.vector.tensor_copy(ot0[:], xsT[0:1, 0, :])
                    nc.sync.dma_start(outd[t], ot0[:])
                    return None
                # ============ radial (f32) ============
                prod = rp.tile([BLK, NBLK, 3, 3], f32, tag="prod")
                nc.vector.tensor_tensor(
                    out=prod[:],
                    in0=gsc[:, :, 4:13].rearrange("p b (i j) -> p b j i", i=3, j=3),
                    in1=shf[:].unsqueeze(2).to_broadcast([BLK, NBLK, 3, 3]),
                    op=OP.mult)
                tvec = rp.tile([BLK, NBLK, 3], f32, tag="tvec")
                nc.vector.tensor_reduce(out=tvec[:], in_=prod[:], axis=AX.X, op=OP.add)
                rv = rp.tile([BLK, NBLK, 3], f32, tag="rv")
                nc.vector.tensor_tensor(out=rv[:], in0=gdc[:, :, 0:3],
                                        in1=gsc[:, :, 0:3], op=OP.subtract)
                nc.vector.tensor_tensor(out=rv[:], in0=rv[:], in1=tvec[:], op=OP.add)
                sq = rp.tile([BLK, NBLK, 3], f32, tag="sq")
                nc.vector.tensor_tensor(out=sq[:], in0=rv[:], in1=rv[:], op=OP.mult)
                d2 = rp.tile([BLK, NBLK], f32, tag="d2")
                nc.vector.tensor_reduce(out=d2[:], in_=sq[:], axis=AX.X, op=OP.add)
                nc.vector.tensor_scalar(out=d2[:], in0=d2[:], scalar1=1e-24,
                                        scalar2=None, op0=OP.max)
                # Newton rsqrt (2 iters)
                sh = rp.tile([BLK, NBLK], i32, tag="sh")
                nc.vector.tensor_scalar(out=sh[:], in0=d2[:].bitcast(i32), scalar1=1,
                                        scalar2=None, op0=OP.arith_shift_right)
                yi = rp.tile([BLK, NBLK], i32, tag="yi")
                nc.vector.tensor_tensor(out=yi[:], in0=magic_t[:], in1=sh[:],
                                        op=OP.subtract)
                y = yi[:].bitcast(f32)
                d2h = rp.tile([BLK, NBLK], f32, tag="d2h")
                nc.vector.tensor_scalar(out=d2h[:], in0=d2[:], scalar1=0.5,
                                        scalar2=None, op0=OP.mult)
                tmp = rp.tile([BLK, NBLK], f32, tag="tmp")
                for _ in range(2):
                    nc.vector.tensor_tensor(out=tmp[:], in0=y, in1=y, op=OP.mult)
                    nc.vector.tensor_tensor(out=tmp[:], in0=tmp[:], in1=d2h[:], op=OP.mult)
                    nc.vector.tensor_scalar(out=tmp[:], in0=tmp[:], scalar1=-1.0,
                                            scalar2=1.5, op0=OP.mult, op1=OP.add)
                    nc.vector.tensor_tensor(out=yi[:].bitcast(f32), in0=y, in1=tmp[:],
                                            op=OP.mult)
                dist = rp.tile([BLK, NBLK], f32, tag="dist")
                nc.vector.tensor_tensor(out=dist[:], in0=d2[:], in1=y, op=OP.mult)
                nc.vector.tensor_scalar(out=dist[:], in0=dist[:], scalar1=1e-6,
                                        scalar2=None, op0=OP.add)
                # r = 1/(dist+1e-6), one NR step from seed y
                nc.vector.tensor_tensor(out=tmp[:], in0=dist[:], in1=y, op=OP.mult)
                nc.vector.tensor_scalar(out=tmp[:], in0=tmp[:], scalar1=-1.0,
                                        scalar2=2.0, op0=OP.mult, op1=OP.add)
                r_ = rp.tile([BLK, NBLK], f32, tag="r_")
                nc.vector.tensor_tensor(out=r_[:], in0=y, in1=tmp[:], op=OP.mult)
                dsc = rp.tile([BLK, NBLK], f32, tag="dsc")
                nc.vector.tensor_scalar(out=dsc[:], in0=dist[:], scalar1=dscale_c,
                                        scalar2=None, op0=OP.mult)

                # ============ embedding (edge-major) ============
                u = ep.tile([BLK, NBLK, NB], f32, tag="u")
                nc.vector.tensor_tensor(
                    out=u[:], in0=r_[:].unsqueeze(2).to_broadcast([BLK, NBLK, NB]),
                    in1=cn_t[:].unsqueeze(1).to_broadcast([BLK, NBLK, NB]), op=OP.mult)
                icv = ep.tile([BLK, NBLK, NB], i16, tag="icv")
                nc.vector.tensor_copy(icv[:], u[:])
                nc.vector.tensor_tensor(out=u[:], in0=u[:], in1=icv[:], op=OP.subtract)
                sinv = ep.tile([BLK, NBLK * NB], f16, tag="sinv")
                sinv_v = sinv[:].rearrange("p (b n) -> p b n", b=NBLK)
                nc.scalar.activation(sinv_v, u[:], AF.Sin, bias=sinb_t[:, 0:1],
                                     scale=TWO_PI)
                for b in range(NBLK):
                    nc.scalar.activation(sinv_v[:, b, :], sinv_v[:, b, :], AF.Copy,
                                         scale=dsc[:, b:b + 1])

                if stage < 2:
                    ot0 = sp.tile([1, ET], f32, tag="ot")
                    nc.vector.tensor_copy(ot0[:], sinv[0:1, 0:ET])
                    nc.sync.dma_start(outd[t], ot0[:])
                    return None
                if stage < 3:
                    ot0 = sp.tile([1, ET], f32, tag="ot")
                    nc.vector.tensor_copy(ot0[:], embT[0:1, 0, :])
                    nc.sync.dma_start(outd[t], ot0[:])
                    return None
                # ============ TP replication + products ============
                prod16 = tp.tile([BLK, 11, ET], f16, tag="prod16")
                psB0 = pgB.tile([BLK, NS], f32, space="PSUM", tag="b")
                nc.tensor.matmul(psB0[:], t0_t[:], xdT[:, 0, :], start=True, stop=True)
                xd0t = tp.tile([BLK, ET], f16, tag="xd0t")
                nc.scalar.copy(xd0t[:], psB0[:])
                for c in range(8):
                    psA = pgA.tile([BLK, NS], f32, space="PSUM", tag="a")
                    nc.tensor.matmul(psA[:], s0_t[:, c * BLK:(c + 1) * BLK],
                                     xsT[:, 0, :], start=True, stop=True)
                    nc.vector.tensor_tensor(out=prod16[:, c, :], in0=psA[:],
                                            in1=xd0t[:], op=OP.mult)
                for i in range(3):
                    psB = pgB.tile([BLK, NS], f32, space="PSUM", tag="b")
                    nc.tensor.matmul(psB[:], t1_t[:, i * BLK:(i + 1) * BLK],
                                     xdT[:, 0, :], start=True, stop=True)
                    bsb = tp.tile([BLK, ET], f16, tag="bsb")
                    nc.scalar.copy(bsb[:], psB[:])
                    for c in range(2):
                        psA = pgA.tile([BLK, NS], f32, space="PSUM", tag="a")
                        nc.tensor.matmul(psA[:], s1_t[:, (c * 3 + i) * BLK:(c * 3 + i + 1) * BLK],
                                         xsT[:, 0, :], start=True, stop=True)
                        if i == 0:
                            nc.vector.tensor_tensor(out=prod16[:, 8 + c, :], in0=psA[:],
                                                    in1=bsb[:], op=OP.mult)
                        else:
                            tmq = tp.tile([BLK, ET], f16, tag="tmq")
                            nc.vector.tensor_tensor(out=tmq[:], in0=psA[:], in1=bsb[:],
                                                    op=OP.mult)
                            nc.vector.tensor_tensor(out=prod16[:, 8 + c, :],
                                                    in0=prod16[:, 8 + c, :],
                                                    in1=tmq[:], op=OP.add)
                for i in range(5):
                    psB = pgB.tile([BLK, NS], f32, space="PSUM", tag="b")
                    nc.tensor.matmul(psB[:64, :], t2_t[:, i * 64:(i + 1) * 64],
                                     xdT[:, 0, :], start=True, stop=True)
                    bsb2 = tp.tile([BLK, ET], f16, tag="bsb2")
                    nc.scalar.copy(bsb2[:64, :], psB[:64, :])
                    psA = pgA.tile([BLK, NS], f32, space="PSUM", tag="a")
                    nc.tensor.matmul(psA[:64, :], s2_t[:, i * 64:(i + 1) * 64],
                                     xsT[:, 0, :], start=True, stop=True)
                    if i == 0:
                        nc.vector.tensor_tensor(out=prod16[:64, 10, :], in0=psA[:64, :],
                                                in1=bsb2[:64, :], op=OP.mult)
                    else:
                        tmq2 = tp.tile([BLK, ET], f16, tag="tmq")
                        nc.vector.tensor_tensor(out=tmq2[:64, :], in0=psA[:64, :],
                                                in1=bsb2[:64, :], op=OP.mult)
                        nc.vector.tensor_tensor(out=prod16[:64, 10, :],
                                                in0=prod16[:64, 10, :],
                                                in1=tmq2[:64, :], op=OP.add)

                if stage < 4:
                    ot0 = sp.tile([1, ET], f32, tag="ot")
                    nc.vector.tensor_copy(ot0[:], prod16[0:1, 0, :])
                    nc.sync.dma_start(outd[t], ot0[:])
                    return None
                # ===== mix einsum: c-order in m-pair passes (overlaps the
                # ===== DVE product chain) + LN stats (one bank: mu@0, s2@64)
                mix_sb = mp.tile([BLK, 4, NS], f16, tag="mix_sb")
                sqf = mp.tile([BLK, 4, NS], f16, tag="sqf")
                mu_ps = pst.tile([BLK, NS], f32, space="PSUM", tag="st")
                s2_ps = pst.tile([BLK, NS], f32, space="PSUM", tag="st")

                def mix_chunk(acc, c, m, start):
                    if c < 8:
                        nc.tensor.matmul(acc[:], w0_t[:, c, m * BLK:(m + 1) * BLK],
                                         prod16[:, c, :], start=start, stop=False)
                    elif c < 10:
                        nc.tensor.matmul(acc[:], w1_t[:, c - 8, m * BLK:(m + 1) * BLK],
                                         prod16[:, c, :], start=start, stop=False)
                    else:
                        nc.tensor.matmul(acc[:], w2_t[:, m * BLK:(m + 1) * BLK],
                                         prod16[:64, 10, :], start=start, stop=True)

                for m in range(4):
                    acc = pacc.tile([BLK, NS], f32, space="PSUM", tag="a")
                    for c in range(11):
                        mix_chunk(acc, c, m, start=(c == 0))
                    nc.scalar.copy(mix_sb[:, m, :], acc[:])
                    nc.vector.tensor_tensor(out=sqf[:, m, :], in0=mix_sb[:, m, :],
                                            in1=mix_sb[:, m, :], op=OP.mult)
                    nc.tensor.matmul(mu_ps[0:1, :], onesc_t[:], mix_sb[:, m, :],
                                     start=(m == 0), stop=(m == 3))
                    nc.tensor.matmul(s2_ps[0:1, :], onesc_t[:], sqf[:, m, :],
                                     start=(m == 0), stop=(m == 3))


                if stage < 5:
                    ot0 = sp.tile([1, ET], f32, tag="ot")
                    nc.vector.tensor_copy(ot0[:], mix_sb[0:1, 0, :])
                    nc.sync.dma_start(outd[t], ot0[:])
                    return None
                # embT via PE transposes (fp16, 1c/row)
                embT = xp.tile([BLK, 2, ET], f16, tag="embT")
                sinb = sinv[:].rearrange("p (b n) -> p b n", b=NBLK)
                for b in range(NBLK):
                    for k in range(2):
                        pt = pgA.tile([BLK, NS], f32, space="PSUM", tag="a")
                        ptv = pt[:].bitcast(f16)[:, 0:BLK]
                        nc.tensor.transpose(ptv,
                                            sinb[:, b, k * BLK:(k + 1) * BLK],
                                            id_t[:])
                        nc.scalar.copy(embT[:, k, b * BLK:(b + 1) * BLK], ptv)

                # ============ df MLP layer 1 (gap filler on PE) ============
                h1c = hp1.tile([BLK, 8, ET], f16, tag="h1c")
                for m in range(8):
                    acc = pacc.tile([BLK, NS], f32, space="PSUM", tag="a")
                    for k in range(2):
                        nc.tensor.matmul(acc[:], dfw1_t[:, k, m * BLK:(m + 1) * BLK],
                                         embT[:, k, :], start=(k == 0), stop=(k == 1))
                    silu_to(h1c[:, m, :], acc[:], bdf1_t[:, m:m + 1])
                # ============ df MLP layer 2 ============
                dff = mp.tile([BLK, 4, NS], f16, tag="dff")
                for m in range(4):
                    acc = pacc.tile([BLK, NS], f32, space="PSUM", tag="a")
                    for k in range(8):
                        nc.tensor.matmul(acc[:], dfw2_t[:, k, m * BLK:(m + 1) * BLK],
                                         h1c[:, k, :], start=(k == 0), stop=(k == 7))
                    nc.scalar.activation(dff[:, m, :], acc[:], AF.Identity,
                                         bias=bdf2_t[:, m:m + 1], scale=1.0)

                # ============ LN row math (f32, partition 0) ============
                rmu = wp.tile([1, NS], f32, tag="rmu")
                nc.scalar.activation(rmu[:], mu_ps[0:1, :], AF.Copy, scale=1.0 / NS)
                rs2 = wp.tile([1, NS], f32, tag="rs2")
                nc.scalar.activation(rs2[:], s2_ps[0:1, :], AF.Copy, scale=1.0 / NS)
                rt = wp.tile([1, NS], f32, tag="rt")
                nc.vector.tensor_tensor(out=rt[:], in0=rmu[:], in1=rmu[:], op=OP.mult)
                rv_ = wp.tile([1, NS], f32, tag="rv_")
                nc.vector.scalar_tensor_tensor(out=rv_[:], in0=rs2[:],
                                               scalar=1e-5, in1=rt[:],
                                               op0=OP.add, op1=OP.subtract)
                nc.vector.tensor_scalar(out=rt[:].bitcast(i32), in0=rv_[:].bitcast(i32),
                                        scalar1=1, scalar2=None,
                                        op0=OP.arith_shift_right)
                ryi = wp.tile([1, NS], i32, tag="ry")
                nc.vector.tensor_tensor(out=ryi[:], in0=magicr_t[:],
                                        in1=rt[:].bitcast(i32), op=OP.subtract)
                ry = ryi[:].bitcast(f32)
                nc.vector.tensor_tensor(out=rt[:], in0=ryi[:].bitcast(f32), in1=ryi[:].bitcast(f32), op=OP.mult)
                nc.vector.scalar_tensor_tensor(out=rt[:], in0=rt[:],
                                               scalar=-0.5, in1=rv_[:],
                                               op0=OP.mult, op1=OP.mult)
                rA2 = wp.tile([1, NS], f32r, tag="rA2")
                nc.vector.scalar_tensor_tensor(out=rA2[:],
                                               in0=rt[:], scalar=1.5,
                                               in1=ryi[:].bitcast(f32),
                                               op0=OP.add, op1=OP.mult)
                rB2 = wp.tile([1, NS], f32r, tag="rB2")
                nc.vector.tensor_tensor(out=rB2[:], in0=rmu[:],
                                        in1=rA2[:].bitcast(f32), op=OP.mult)

                # broadcasts into the replication banks, then to SBUF fp16
                # (frees the banks early and makes the reg chain all-2x)
                Abc = pacc.tile([BLK, NS], f32, space="PSUM", tag="a")
                nc.tensor.matmul(Abc[:], onesr_t[0:1, :].bitcast(f32r),
                                 rA2[:], start=True, stop=True)
                Bbc = pacc.tile([BLK, NS], f32, space="PSUM", tag="a")
                nc.tensor.matmul(Bbc[:], onesr_t[0:1, :].bitcast(f32r),
                                 rB2[:], start=True, stop=True)
                Asb = mp.tile([BLK, NS], f16, tag="Asb")
                nc.scalar.copy(Asb[:], Abc[:])
                Bsb = mp.tile([BLK, NS], f16, tag="Bsb")
                nc.scalar.copy(Bsb[:], Bbc[:])

                # ============ reg = (mix - mu)*rstd*g (*) df ============
                for k in range(4):
                    nc.vector.tensor_tensor(out=sqf[:, k, :], in0=mix_sb[:, k, :],
                                            in1=Asb[:], op=OP.mult)
                    nc.vector.tensor_tensor(out=sqf[:, k, :], in0=sqf[:, k, :],
                                            in1=Bsb[:], op=OP.subtract)
                    nc.vector.tensor_tensor(out=dff[:, k, :], in0=sqf[:, k, :],
                                            in1=dff[:, k, :], op=OP.mult)
                reg = dff
                return dict(reg=dff)

            def back(t, fs):
                reg = fs["reg"]


                # ============ mix MLP ============
                h = hp.tile([BLK, 8, ET], f16, tag="h")
                for m in range(8):
                    acc = pacc.tile([BLK, NS], f32, space="PSUM", tag="a")
                    for k in range(4):
                        nc.tensor.matmul(acc[:], miw1_t[:, k, m * BLK:(m + 1) * BLK],
                                         reg[:, k, :], start=(k == 0), stop=(k == 3))
                    silu_to(h[:, m, :], acc[:], bmi1_t[:, m:m + 1])
                po = pgB.tile([BLK, NS], f32, space="PSUM", tag="b")
                for m in range(8):
                    acc = pacc.tile([BLK, NS], f32, space="PSUM", tag="a")
                    for k in range(8):
                        nc.tensor.matmul(acc[:], miw2_t[:, k, m * BLK:(m + 1) * BLK],
                                         h[:, k, :], start=(k == 0), stop=(k == 7))
                    h2m = sp.tile([BLK, ET], f16, tag="h2m")
                    silu_to(h2m[:], acc[:], bmi2_t[:, m:m + 1])
                    nc.tensor.matmul(po[0:1, :], mow_t[:, m:m + 1], h2m[:],
                                     start=(m == 0), stop=(m == 7))
                ot = sp.tile([1, ET], f32, tag="ot")
                nc.scalar.activation(ot[:], po[0:1, :], AF.Identity, bias=bmo_t[:, 0:1],
                                     scale=1.0)
                nc.sync.dma_start(outd[t], ot[:])

            # 2-stage software pipeline: FRONT(t+1) is emitted before
            # BACK(t) so the scheduler always has high-priority PE work
            # during BACK's serial LN/reg chains.
            pend = None
            for t in [tt for _ in range(reps) for tt in range(ntiles)]:
                fs = front(t)
                if fs is None:
                    continue
                if pend is not None:
                    back(*pend)
                pend = (t, fs)
            if pend is not None:
                back(*pend)

    nc.finalize()
    return nc


def _wrap16(v):
    """dma_gather index layout: idx i at [i % 16, i // 16], replicated to
    128 partitions (8 gpsimd cores x 16)."""
    n = v.shape[-1]
    w = np.zeros(v.shape[:-1] + (128, n // 16), np.int16)
    r = v.reshape(v.shape[:-1] + (n // 16, 16))
    for rep in range(8):
        w[..., 16 * rep:16 * (rep + 1), :] = np.swapaxes(r, -1, -2)
    return w


def _host_prep(inputs):
    """Shared (replicated) host-side tensors."""
    f = np.float32
    nodes = np.asarray(inputs["nodes"], f)
    pos = np.asarray(inputs["pos"], f)
    cell = np.asarray(inputs["cell"], f)
    W0 = np.asarray(inputs["W0"], f)
    W1 = np.asarray(inputs["W1"], f)
    W2 = np.asarray(inputs["W2"], f)
    ln_g = np.asarray(inputs["ln_g"], f)

    nodesF = np.zeros((N, BLK), np.float16)
    nodesF[:, :FEAT] = nodes.astype(np.float16)
    bv = np.asarray(inputs["batch_vec"]).astype(np.int64)
    posC = np.zeros((N, 16), f)
    posC[:, :3] = pos
    posC[:, 4:13] = cell.reshape(G, 9)[bv]

    sym = lambda W: 0.5 * (W + W.transpose(1, 0, 2))
    w0f = (sym(W0) / FAN).reshape(L0 * L0, NS)
    w1f = (sym(W1) / (FAN * math.sqrt(3.0))).reshape(L1 * L1, NS)
    w2f = (sym(W2) / (FAN * math.sqrt(5.0))).reshape(L2 * L2, NS)
    h16 = np.float16

    def chunk(w, nch):
        return np.ascontiguousarray(
            w.reshape(nch, BLK, w.shape[1]).transpose(1, 0, 2)).astype(h16)

    miw1 = ln_g[:, None] * np.asarray(inputs["mi_w1"], f)

    def colbias(b, nch):
        b = np.asarray(b, f).reshape(nch, BLK)
        return np.ascontiguousarray(b.T)

    O0, O1, O2 = 0, L0, L0 + 3 * L1
    s0 = np.zeros((BLK, 8 * BLK), h16)
    for c in range(8):
        for p in range(BLK):
            s0[O0 + c * 4 + p // 32, c * BLK + p] = 1.0
    t0 = np.zeros((BLK, BLK), h16)
    for p in range(BLK):
        t0[O0 + p % 32, p] = 1.0
    s1 = np.zeros((BLK, 6 * BLK), h16)
    for c in range(2):
        for i in range(3):
            for p in range(BLK):
                u = c * 8 + p // 16
                s1[O1 + u * 3 + i, (c * 3 + i) * BLK + p] = 1.0
    t1 = np.zeros((BLK, 3 * BLK), h16)
    for i in range(3):
        for p in range(BLK):
            t1[O1 + (p % 16) * 3 + i, i * BLK + p] = 1.0
    s2 = np.zeros((BLK, 5 * 64), h16)
    t2 = np.zeros((BLK, 5 * 64), h16)
    for i in range(5):
        for p in range(64):
            s2[O2 + (p // 8) * 5 + i, i * 64 + p] = 1.0
            t2[O2 + (p % 8) * 5 + i, i * 64 + p] = 1.0
    cn = np.broadcast_to((np.arange(1, NB + 1, dtype=f) / (2.0 * CUT))[None, :],
                         (BLK, NB)).copy()
    return dict(
        nodesF=nodesF, posC=posC,
        w0p=chunk(w0f, 8), w1p=chunk(w1f, 2),
        w2p=np.ascontiguousarray(w2f).astype(h16),
        dfw1p=chunk(np.asarray(inputs["df_w1"], f), 2),
        dfw2p=chunk(np.asarray(inputs["df_w2"], f), 8),
        miw1p=chunk(miw1, 4),
        miw2p=chunk(np.asarray(inputs["mi_w2"], f), 8),
        mowp=np.ascontiguousarray(
            np.asarray(inputs["mo_w"], f).reshape(8, BLK).T).astype(h16),
        s0d=s0, t0d=t0, s1d=s1, t1d=t1, s2d=s2, t2d=t2,
        bdf1=colbias(inputs["df_b1"], 8), bdf2=colbias(inputs["df_b2"], 4),
        bmi1=colbias(inputs["mi_b1"], 8), bmi2=colbias(inputs["mi_b2"], 8),
        bmo=np.asarray(inputs["mo_b"], f).reshape(1, 1),
        onesrd=np.ones((1, BLK), f), cnd=cn,
        identd=np.eye(BLK, dtype=h16),
    )


def _edge_prep(inputs, core, ntiles):
    """Per-core edge tensors."""
    f = np.float32
    ec = ntiles * ET
    lo = core * EC
    ei = np.asarray(inputs["edge_index"])
    src = ei[0, lo:lo + ec].astype(np.int32)
    dst = ei[1, lo:lo + ec].astype(np.int32)
    shift = np.asarray(inputs["edge_shift"], f)[lo:lo + ec]

    def tile_idx(x):
        return np.ascontiguousarray(x.reshape(ntiles, NBLK, BLK).transpose(0, 2, 1))

    pcidx = np.concatenate([tile_idx(src), tile_idx(dst)], axis=2)

    return dict(
        g16s=_wrap16(src.reshape(ntiles, ET).astype(np.int16)),
        g16d=_wrap16(dst.reshape(ntiles, ET).astype(np.int16)),
        pcidx=pcidx,
        shiftd=np.ascontiguousarray(
            shift.reshape(ntiles, NBLK, BLK, 3).transpose(0, 2, 1, 3)),
    )


def _run(inputs, mode, ntiles, ncores):
    key = (mode, ntiles, 1)
    if key not in _cache:
        _cache[key] = _build(mode, ntiles)
    nc = _cache[key]
    shared = _host_prep(inputs)
    in_maps = []
    for c in range(ncores):
        m = dict(shared)
        m.update(_edge_prep(inputs, c, ntiles))
        in_maps.append(m)

    if mode == "sim":
        from concourse.bass_interp import CoreSim
        outs = []
        for c in range(ncores):
            sim = CoreSim(nc)
            for k, v in in_maps[c].items():
                sim.tensor(k)[:] = v
            sim.simulate()
            outs.append(np.array(sim.tensor("out")).reshape(-1))
        return np.concatenate(outs).reshape(-1, 1)

    from concourse.bass_utils import run_bass_kernel_spmd
    trace = os.environ.get("EXB_TRACE", "0") == "1"
    res = run_bass_kernel_spmd(nc, in_maps, list(range(ncores)), trace=trace)
    out = np.concatenate([res.results[c]["out"].reshape(-1) for c in range(ncores)])
    if trace:
        _run.last_exec_time_ns = res.exec_time_ns
    return out.reshape(-1, 1)


def kernel(**inputs) -> np.ndarray:
    return _run(inputs, os.environ.get("EXB_MODE", "hw"), EC // ET, NCORES).astype(np.float32)



# revision 2
# speedup vs baseline: 1498.4432x; 1498.4432x over previous
"""Trainium2 Bass kernel for nn_ExchangeBlock (gnn_message_passing).

Data-parallel over edges: each of the 8 cores processes E/8 = 16384 edges,
node features and weights replicated.  Per 512-edge tile:
  - node features gathered FEATURE-MAJOR via transposing dma_gather (fp16,
    no PE transposes); pos/cell rows via classic indirect DMA
  - radial: tvec, dist (DVE Newton rsqrt), Bessel embedding (range-reduced
    Sin on ACT); embedding transposed to feature-major with DMA XBAR
    transposes (no PE time)
  - fp16 matmuls (free=512 -> 1 cycle/row) for the distance-filter MLP, the
    symmetrized tensor product and the mix MLP.  All stationary operands are
    padded to 128 columns so FWL stays enabled (l2 path zero-padded to the
    full 128 partitions; LN stats via an all-ones [128,128] stationary;
    final mo dot via a column-padded [128,128] stationary).  LayerNorm
    stats run as full-width [128,512] chains on DVE (Newton rsqrt), which
    produces the broadcast A/B tiles directly - no PE broadcast matmuls.
All activations/weights fp16 (quantization ~1e-3), radial + LN-stat math
f32.  PSUM banks: 2 replication A, 2 B/output-row, 3 accumulation
rotation, 1 stats (mu then sq, sequenced).
"""
import os
import sys

sys.path.insert(0, "/opt/trn_rl_repo")

import math
import numpy as np

L0, L1, L2 = 32, 16, 8
NS, NB = 512, 256
CUT = 7.0
N, E, G = 16384, 131072, 16
FEAT = L0 + 3 * L1 + 5 * L2  # 120
NCORES = 8
EC = E // NCORES  # edges per core
BLK = 128
ET = 512  # edges per tile (= one PSUM bank of fp32)
NBLK = ET // BLK
FAN = math.sqrt(float(L0 * L0 + L1 * L1 + L2 * L2))
EMBC = math.sqrt(2.0 / CUT)
MAGIC = 0x5F3759DF

_cache = {}


def _build(mode, ntiles, reps=1, stage=99):
    """Build the Bass program (shared by all cores, SPMD)."""
    import concourse.bacc as bacc
    import concourse.bass as bass
    import concourse.mybir as mybir
    import concourse.tile as tile

    f32 = mybir.dt.float32
    f32r = mybir.dt.float32r
    f16 = mybir.dt.float16
    i32 = mybir.dt.int32
    i16 = mybir.dt.int16
    AF = mybir.ActivationFunctionType
    OP = mybir.AluOpType
    AX = mybir.AxisListType

    nc = bacc.Bacc(None)

    # ---------------- DRAM tensors ----------------
    nodesF = nc.dram_tensor("nodesF", [N, BLK], f16, kind="ExternalInput")
    posC = nc.dram_tensor("posC", [N, 16], f32, kind="ExternalInput")
    g16s = nc.dram_tensor("g16s", [ntiles, BLK, ET // 16], i16, kind="ExternalInput")
    g16d = nc.dram_tensor("g16d", [ntiles, BLK, ET // 16], i16, kind="ExternalInput")
    pcidx = nc.dram_tensor("pcidx", [ntiles, BLK, 8], i32, kind="ExternalInput")
    shiftd = nc.dram_tensor("shiftd", [ntiles, BLK, NBLK, 3], f32, kind="ExternalInput")

    w0p = nc.dram_tensor("w0p", [BLK, 8, NS], f16, kind="ExternalInput")
    w1p = nc.dram_tensor("w1p", [BLK, 2, NS], f16, kind="ExternalInput")
    w2p = nc.dram_tensor("w2p", [BLK, NS], f16, kind="ExternalInput")
    dfw1p = nc.dram_tensor("dfw1p", [BLK, 2, 1024], f16, kind="ExternalInput")
    dfw2p = nc.dram_tensor("dfw2p", [BLK, 8, NS], f16, kind="ExternalInput")
    miw1p = nc.dram_tensor("miw1p", [BLK, 4, 1024], f16, kind="ExternalInput")
    miw2p = nc.dram_tensor("miw2p", [BLK, 8, 1024], f16, kind="ExternalInput")
    mowp = nc.dram_tensor("mowp", [BLK, 8, BLK], f16, kind="ExternalInput")
    s0d = nc.dram_tensor("s0d", [BLK, 8 * BLK], f16, kind="ExternalInput")
    t0d = nc.dram_tensor("t0d", [BLK, BLK], f16, kind="ExternalInput")
    s1d = nc.dram_tensor("s1d", [BLK, 6 * BLK], f16, kind="ExternalInput")
    t1d = nc.dram_tensor("t1d", [BLK, 3 * BLK], f16, kind="ExternalInput")
    s2d = nc.dram_tensor("s2d", [BLK, 5 * BLK], f16, kind="ExternalInput")
    t2d = nc.dram_tensor("t2d", [BLK, 5 * BLK], f16, kind="ExternalInput")
    bdf1 = nc.dram_tensor("bdf1", [BLK, 8], f32, kind="ExternalInput")
    bdf2 = nc.dram_tensor("bdf2", [BLK, 4], f32, kind="ExternalInput")
    bmi1 = nc.dram_tensor("bmi1", [BLK, 8], f32, kind="ExternalInput")
    bmi2 = nc.dram_tensor("bmi2", [BLK, 8], f32, kind="ExternalInput")
    bmo = nc.dram_tensor("bmo", [1, 1], f32, kind="ExternalInput")
    cnd = nc.dram_tensor("cnd", [BLK, NB], f32, kind="ExternalInput")

    outd = nc.dram_tensor("out", [ntiles, 1, ET], f32, kind="ExternalOutput")

    TWO_PI = 2.0 * math.pi
    sin_bias = -math.pi if mode == "sim" else 0.0
    dscale_c = -EMBC if mode == "sim" else EMBC

    with tile.TileContext(nc) as tc:
        with (
            tc.tile_pool(name="const", bufs=1) as cp,
            tc.tile_pool(name="gat", bufs=3) as gp,
            tc.tile_pool(name="rad", bufs=3) as rp,
            tc.tile_pool(name="emb", bufs=2) as ep,
            tc.tile_pool(name="trx", bufs=3) as xp,
            tc.tile_pool(name="tpp", bufs=2) as tp,
            tc.tile_pool(name="mid", bufs=2) as mp,
            tc.tile_pool(name="row", bufs=1) as wp,
            tc.tile_pool(name="h1s", bufs=2) as hp1,
            tc.tile_pool(name="hs", bufs=2) as hp,
            tc.tile_pool(name="sml", bufs=2) as sp,
            tc.tile_pool(name="psA", bufs=2, space="PSUM") as pgA,
            tc.tile_pool(name="psB", bufs=2, space="PSUM") as pgB,
            tc.tile_pool(name="pac", bufs=3, space="PSUM") as pacc,
            tc.tile_pool(name="pst", bufs=1, space="PSUM") as pst,
        ):
            # ---------------- constants ----------------
            w0_t = cp.tile([BLK, 8, NS], f16)
            nc.sync.dma_start(w0_t[:], w0p[:])
            w1_t = cp.tile([BLK, 2, NS], f16)
            nc.sync.dma_start(w1_t[:], w1p[:])
            w2_t = cp.tile([BLK, NS], f16)
            nc.sync.dma_start(w2_t[:], w2p[:])
            dfw1_t = cp.tile([BLK, 2, 1024], f16)
            nc.sync.dma_start(dfw1_t[:], dfw1p[:])
            dfw2_t = cp.tile([BLK, 8, NS], f16)
            nc.sync.dma_start(dfw2_t[:], dfw2p[:])
            miw1_t = cp.tile([BLK, 4, 1024], f16)
            nc.sync.dma_start(miw1_t[:], miw1p[:])
            miw2_t = cp.tile([BLK, 8, 1024], f16)
            nc.sync.dma_start(miw2_t[:], miw2p[:])
            mow_t = cp.tile([BLK, 8, BLK], f16)
            nc.sync.dma_start(mow_t[:], mowp[:])
            s0_t = cp.tile([BLK, 8 * BLK], f16)
            nc.sync.dma_start(s0_t[:], s0d[:])
            t0_t = cp.tile([BLK, BLK], f16)
            nc.sync.dma_start(t0_t[:], t0d[:])
            s1_t = cp.tile([BLK, 6 * BLK], f16)
            nc.sync.dma_start(s1_t[:], s1d[:])
            t1_t = cp.tile([BLK, 3 * BLK], f16)
            nc.sync.dma_start(t1_t[:], t1d[:])
            s2_t = cp.tile([BLK, 5 * BLK], f16)
            nc.sync.dma_start(s2_t[:], s2d[:])
            t2_t = cp.tile([BLK, 5 * BLK], f16)
            nc.sync.dma_start(t2_t[:], t2d[:])
            bdf1_t = cp.tile([BLK, 8], f32)
            nc.sync.dma_start(bdf1_t[:], bdf1[:])
            bdf2_t = cp.tile([BLK, 4], f32)
            nc.sync.dma_start(bdf2_t[:], bdf2[:])
            bmi1_t = cp.tile([BLK, 8], f32)
            nc.sync.dma_start(bmi1_t[:], bmi1[:])
            bmi2_t = cp.tile([BLK, 8], f32)
            nc.sync.dma_start(bmi2_t[:], bmi2[:])
            bmo_t = cp.tile([1, 1], f32)
            nc.sync.dma_start(bmo_t[:], bmo[:])
            cn_t = cp.tile([BLK, NB], f32)
            nc.sync.dma_start(cn_t[:], cnd[:])
            ones128_t = cp.tile([BLK, BLK], f16)
            nc.gpsimd.memset(ones128_t[:], 1.0)
            magic_t = cp.tile([BLK, NBLK], i32)
            nc.gpsimd.memset(magic_t[:], MAGIC)
            sinb_t = cp.tile([BLK, 1], f32)
            nc.gpsimd.memset(sinb_t[:], sin_bias)

            def silu_to(dst, ps, bias_ap):
                if mode == "sim":
                    sg = sp.tile([BLK, ET], f16, tag="sg")
                    nc.scalar.activation(sg[:], ps, AF.Sigmoid, bias=bias_ap, scale=1.0)
                    pre = sp.tile([BLK, ET], f16, tag="pre")
                    nc.vector.tensor_scalar(out=pre[:], in0=ps, scalar1=bias_ap,
                                            scalar2=None, op0=OP.add)
                    nc.vector.tensor_tensor(out=dst, in0=sg[:], in1=pre[:], op=OP.mult)
                else:
                    nc.scalar.activation(dst, ps, AF.Silu, bias=bias_ap, scale=1.0)

            def front(t):
                # ============ index DMAs ============
                if stage < 0:
                    return None
                gs16 = gp.tile([BLK, ET // 16], i16, tag="gs16")
                gd16 = gp.tile([BLK, ET // 16], i16, tag="gd16")
                nc.sync.dma_start(gs16[:], g16s[t])
                nc.sync.dma_start(gd16[:], g16d[t])
                pci = gp.tile([BLK, 8], i32, tag="pci")
                nc.sync.dma_start(pci[:], pcidx[t])
                shf = gp.tile([BLK, NBLK, 3], f32, tag="shf")
                nc.sync.dma_start(shf[:], shiftd[t])

                # ============ gathers ============
                xsT = xp.tile([BLK, 1, ET], f16, tag="xsT")
                nc.gpsimd.dma_gather(
                    out_ap=xsT[:], in_ap=nodesF[:], idxs_ap=gs16[:],
                    num_idxs=ET, num_idxs_reg=ET, elem_size=BLK, transpose=True)
                xdT = xp.tile([BLK, 1, ET], f16, tag="xdT")
                nc.gpsimd.dma_gather(
                    out_ap=xdT[:], in_ap=nodesF[:], idxs_ap=gd16[:],
                    num_idxs=ET, num_idxs_reg=ET, elem_size=BLK, transpose=True)
                gsc = gp.tile([BLK, NBLK, 16], f32, tag="gsc")
                gdc = gp.tile([BLK, NBLK, 16], f32, tag="gdc")
                for b in range(NBLK):
                    nc.gpsimd.indirect_dma_start(
                        out=gsc[:, b, :], out_offset=None, in_=posC[:],
                        in_offset=bass.IndirectOffsetOnAxis(ap=pci[:, b:b + 1], axis=0))
                    nc.gpsimd.indirect_dma_start(
                        out=gdc[:, b, :], out_offset=None, in_=posC[:],
                        in_offset=bass.IndirectOffsetOnAxis(ap=pci[:, 4 + b:5 + b], axis=0))

                if stage < 1:
                    ot0 = sp.tile([1, ET], f32, tag="ot")
                    nc.vector.tensor_copy(ot0[:], xsT[0:1, 0, :])
                    nc.sync.dma_start(outd[t], ot0[:])
                    return None
                # ============ radial (f32) ============
                prod = rp.tile([BLK, NBLK, 3, 3], f32, tag="prod")
                nc.vector.tensor_tensor(
                    out=prod[:],
                    in0=gsc[:, :, 4:13].rearrange("p b (i j) -> p b j i", i=3, j=3),
                    in1=shf[:].unsqueeze(2).to_broadcast([BLK, NBLK, 3, 3]),
                    op=OP.mult)
                tvec = rp.tile([BLK, NBLK, 3], f32, tag="tvec")
                nc.vector.tensor_reduce(out=tvec[:], in_=prod[:], axis=AX.X, op=OP.add)
                rv = rp.tile([BLK, NBLK, 3], f32, tag="rv")
                nc.vector.tensor_tensor(out=rv[:], in0=gdc[:, :, 0:3],
                                        in1=gsc[:, :, 0:3], op=OP.subtract)
                nc.vector.tensor_tensor(out=rv[:], in0=rv[:], in1=tvec[:], op=OP.add)
                sq = rp.tile([BLK, NBLK, 3], f32, tag="sq")
                nc.vector.tensor_tensor(out=sq[:], in0=rv[:], in1=rv[:], op=OP.mult)
                d2 = rp.tile([BLK, NBLK], f32, tag="d2")
                nc.vector.tensor_reduce(out=d2[:], in_=sq[:], axis=AX.X, op=OP.add)
                nc.vector.tensor_scalar(out=d2[:], in0=d2[:], scalar1=1e-24,
                                        scalar2=None, op0=OP.max)
                # Newton rsqrt (2 iters)
                sh = rp.tile([BLK, NBLK], i32, tag="sh")
                nc.vector.tensor_scalar(out=sh[:], in0=d2[:].bitcast(i32), scalar1=1,
                                        scalar2=None, op0=OP.arith_shift_right)
                yi = rp.tile([BLK, NBLK], i32, tag="yi")
                nc.vector.tensor_tensor(out=yi[:], in0=magic_t[:], in1=sh[:],
                                        op=OP.subtract)
                y = yi[:].bitcast(f32)
                d2h = rp.tile([BLK, NBLK], f32, tag="d2h")
                nc.vector.tensor_scalar(out=d2h[:], in0=d2[:], scalar1=0.5,
                                        scalar2=None, op0=OP.mult)
                tmp = rp.tile([BLK, NBLK], f32, tag="tmp")
                for _ in range(2):
                    nc.vector.tensor_tensor(out=tmp[:], in0=y, in1=y, op=OP.mult)
                    nc.vector.tensor_tensor(out=tmp[:], in0=tmp[:], in1=d2h[:], op=OP.mult)
                    nc.vector.tensor_scalar(out=tmp[:], in0=tmp[:], scalar1=-1.0,
                                            scalar2=1.5, op0=OP.mult, op1=OP.add)
                    nc.vector.tensor_tensor(out=yi[:].bitcast(f32), in0=y, in1=tmp[:],
                                            op=OP.mult)
                dist = rp.tile([BLK, NBLK], f32, tag="dist")
                nc.vector.tensor_tensor(out=dist[:], in0=d2[:], in1=y, op=OP.mult)
                nc.vector.tensor_scalar(out=dist[:], in0=dist[:], scalar1=1e-6,
                                        scalar2=None, op0=OP.add)
                # r = 1/(dist+1e-6), one NR step from seed y
                nc.vector.tensor_tensor(out=tmp[:], in0=dist[:], in1=y, op=OP.mult)
                nc.vector.tensor_scalar(out=tmp[:], in0=tmp[:], scalar1=-1.0,
                                        scalar2=2.0, op0=OP.mult, op1=OP.add)
                r_ = rp.tile([BLK, NBLK], f32, tag="r_")
                nc.vector.tensor_tensor(out=r_[:], in0=y, in1=tmp[:], op=OP.mult)
                dsc = rp.tile([BLK, NBLK], f32, tag="dsc")
                nc.vector.tensor_scalar(out=dsc[:], in0=dist[:], scalar1=dscale_c,
                                        scalar2=None, op0=OP.mult)

                # ============ embedding (edge-major) ============
                u = ep.tile([BLK, NBLK, NB], f32, tag="u")
                nc.vector.tensor_tensor(
                    out=u[:], in0=r_[:].unsqueeze(2).to_broadcast([BLK, NBLK, NB]),
                    in1=cn_t[:].unsqueeze(1).to_broadcast([BLK, NBLK, NB]), op=OP.mult)
                icv = ep.tile([BLK, NBLK, NB], i16, tag="icv")
                nc.vector.tensor_copy(icv[:], u[:])
                nc.vector.tensor_tensor(out=u[:], in0=u[:], in1=icv[:], op=OP.subtract)
                sinv = ep.tile([BLK, NBLK * NB], f16, tag="sinv")
                sinv_v = sinv[:].rearrange("p (b n) -> p b n", b=NBLK)
                nc.scalar.activation(sinv_v, u[:], AF.Sin, bias=sinb_t[:, 0:1],
                                     scale=TWO_PI)
                for b in range(NBLK):
                    nc.scalar.activation(sinv_v[:, b, :], sinv_v[:, b, :], AF.Copy,
                                         scale=dsc[:, b:b + 1])

                # embT via DMA XBAR transposes (runs on DMA engines, no PE)
                embT = xp.tile([BLK, 2, ET], f16, tag="embT")
                sinb = sinv[:].rearrange("p (b n) -> p b n", b=NBLK)
                for b in range(NBLK):
                    for k in range(2):
                        nc.sync.dma_start_transpose(
                            embT[:, k, b * BLK:(b + 1) * BLK],
                            sinb[:, b, k * BLK:(k + 1) * BLK])

                if stage < 2:
                    ot0 = sp.tile([1, ET], f32, tag="ot")
                    nc.vector.tensor_copy(ot0[:], sinv[0:1, 0:ET])
                    nc.sync.dma_start(outd[t], ot0[:])
                    return None
                if stage < 3:
                    ot0 = sp.tile([1, ET], f32, tag="ot")
                    nc.vector.tensor_copy(ot0[:], embT[0:1, 0, :])
                    nc.sync.dma_start(outd[t], ot0[:])
                    return None
                # ============ TP replication + products ============
                prod16 = tp.tile([BLK, 11, ET], f16, tag="prod16")
                psB0 = pgB.tile([BLK, NS], f32, space="PSUM", tag="b")
                nc.tensor.matmul(psB0[:], t0_t[:], xdT[:, 0, :], start=True, stop=True)
                xd0t = tp.tile([BLK, ET], f16, tag="xd0t")
                nc.scalar.copy(xd0t[:], psB0[:])
                for c in range(8):
                    psA = pgA.tile([BLK, NS], f32, space="PSUM", tag="a")
                    nc.tensor.matmul(psA[:], s0_t[:, c * BLK:(c + 1) * BLK],
                                     xsT[:, 0, :], start=True, stop=True)
                    nc.vector.tensor_tensor(out=prod16[:, c, :], in0=psA[:],
                                            in1=xd0t[:], op=OP.mult)
                for i in range(3):
                    psB = pgB.tile([BLK, NS], f32, space="PSUM", tag="b")
                    nc.tensor.matmul(psB[:], t1_t[:, i * BLK:(i + 1) * BLK],
                                     xdT[:, 0, :], start=True, stop=True)
                    bsb = tp.tile([BLK, ET], f16, tag="bsb")
                    nc.scalar.copy(bsb[:], psB[:])
                    for c in range(2):
                        psA = pgA.tile([BLK, NS], f32, space="PSUM", tag="a")
                        nc.tensor.matmul(psA[:], s1_t[:, (c * 3 + i) * BLK:(c * 3 + i + 1) * BLK],
                                         xsT[:, 0, :], start=True, stop=True)
                        if i == 0:
                            nc.vector.tensor_tensor(out=prod16[:, 8 + c, :], in0=psA[:],
                                                    in1=bsb[:], op=OP.mult)
                        else:
                            tmq = tp.tile([BLK, ET], f16, tag="tmq")
                            nc.vector.tensor_tensor(out=tmq[:], in0=psA[:], in1=bsb[:],
                                                    op=OP.mult)
                            nc.vector.tensor_tensor(out=prod16[:, 8 + c, :],
                                                    in0=prod16[:, 8 + c, :],
                                                    in1=tmq[:], op=OP.add)
                for i in range(5):
                    psB = pgB.tile([BLK, NS], f32, space="PSUM", tag="b")
                    nc.tensor.matmul(psB[:], t2_t[:, i * BLK:(i + 1) * BLK],
                                     xdT[:, 0, :], start=True, stop=True)
                    bsb2 = tp.tile([BLK, ET], f16, tag="bsb2")
                    nc.scalar.copy(bsb2[:], psB[:])
                    psA = pgA.tile([BLK, NS], f32, space="PSUM", tag="a")
                    nc.tensor.matmul(psA[:], s2_t[:, i * BLK:(i + 1) * BLK],
                                     xsT[:, 0, :], start=True, stop=True)
                    if i == 0:
                        nc.vector.tensor_tensor(out=prod16[:, 10, :], in0=psA[:],
                                                in1=bsb2[:], op=OP.mult)
                    else:
                        tmq2 = tp.tile([BLK, ET], f16, tag="tmq")
                        nc.vector.tensor_tensor(out=tmq2[:], in0=psA[:],
                                                in1=bsb2[:], op=OP.mult)
                        nc.vector.tensor_tensor(out=prod16[:, 10, :],
                                                in0=prod16[:, 10, :],
                                                in1=tmq2[:], op=OP.add)

                if stage < 4:
                    ot0 = sp.tile([1, ET], f32, tag="ot")
                    nc.vector.tensor_copy(ot0[:], prod16[0:1, 0, :])
                    nc.sync.dma_start(outd[t], ot0[:])
                    return None
                # ===== mix einsum (c-order in m passes) + LN mu stats =====
                mix_sb = mp.tile([BLK, 4, NS], f16, tag="mix_sb")
                sqf = mp.tile([BLK, 4, NS], f16, tag="sqf")
                mu_ps = pst.tile([BLK, NS], f32, space="PSUM", tag="st")

                def mix_chunk(acc, c, m, start):
                    if c < 8:
                        nc.tensor.matmul(acc[:], w0_t[:, c, m * BLK:(m + 1) * BLK],
                                         prod16[:, c, :], start=start, stop=False)
                    elif c < 10:
                        nc.tensor.matmul(acc[:], w1_t[:, c - 8, m * BLK:(m + 1) * BLK],
                                         prod16[:, c, :], start=start, stop=False)
                    else:
                        nc.tensor.matmul(acc[:], w2_t[:, m * BLK:(m + 1) * BLK],
                                         prod16[:, 10, :], start=start, stop=True)

                for m in range(4):
                    acc = pacc.tile([BLK, NS], f32, space="PSUM", tag="a")
                    for c in range(11):
                        mix_chunk(acc, c, m, start=(c == 0))
                    nc.scalar.copy(mix_sb[:, m, :], acc[:])
                    nc.vector.tensor_tensor(out=sqf[:, m, :], in0=mix_sb[:, m, :],
                                            in1=mix_sb[:, m, :], op=OP.mult)
                    nc.tensor.matmul(mu_ps[:], ones128_t[:], mix_sb[:, m, :],
                                     start=(m == 0), stop=(m == 3))

                if stage < 5:
                    ot0 = sp.tile([1, ET], f32, tag="ot")
                    nc.vector.tensor_copy(ot0[:], mix_sb[0:1, 0, :])
                    nc.sync.dma_start(outd[t], ot0[:])
                    return None

                # mu consumed early so the stats bank can be reused for sq
                rmu = wp.tile([BLK, NS], f32, tag="rmu")
                nc.vector.tensor_scalar(out=rmu[:], in0=mu_ps[:], scalar1=1.0 / NS,
                                        scalar2=None, op0=OP.mult)
                s2_ps = pst.tile([BLK, NS], f32, space="PSUM", tag="st")

                # ============ df MLP layer 1 (+ sq stats interleaved) ======
                h1c = hp1.tile([BLK, 8, ET], f16, tag="h1c")
                for m in range(8):
                    acc = pacc.tile([BLK, NS], f32, space="PSUM", tag="a")
                    for k in range(2):
                        nc.tensor.matmul(acc[:], dfw1_t[:, k, m * BLK:(m + 1) * BLK],
                                         embT[:, k, :], start=(k == 0), stop=(k == 1))
                    silu_to(h1c[:, m, :], acc[:], bdf1_t[:, m:m + 1])
                    if m < 4:
                        nc.tensor.matmul(s2_ps[:], ones128_t[:], sqf[:, m, :],
                                         start=(m == 0), stop=(m == 3))
                # ============ df MLP layer 2 ============
                dff = mp.tile([BLK, 4, NS], f16, tag="dff")
                for m in range(4):
                    acc = pacc.tile([BLK, NS], f32, space="PSUM", tag="a")
                    for k in range(8):
                        nc.tensor.matmul(acc[:], dfw2_t[:, k, m * BLK:(m + 1) * BLK],
                                         h1c[:, k, :], start=(k == 0), stop=(k == 7))
                    nc.scalar.activation(dff[:, m, :], acc[:], AF.Identity,
                                         bias=bdf2_t[:, m:m + 1], scale=1.0)

                # ============ LN chain, full [128, NS] width (f32) ========
                rt = wp.tile([BLK, NS], f32, tag="rt")
                nc.vector.tensor_tensor(out=rt[:], in0=rmu[:], in1=rmu[:], op=OP.mult)
                rvv = wp.tile([BLK, NS], f32, tag="rvv")
                nc.vector.scalar_tensor_tensor(out=rvv[:], in0=s2_ps[:],
                                               scalar=1.0 / NS, in1=rt[:],
                                               op0=OP.mult, op1=OP.subtract)
                nc.vector.tensor_scalar(out=rvv[:], in0=rvv[:], scalar1=1e-5,
                                        scalar2=None, op0=OP.add)
                shv = wp.tile([BLK, NS], i32, tag="shv")
                nc.vector.tensor_scalar(out=shv[:], in0=rvv[:].bitcast(i32), scalar1=1,
                                        scalar2=None, op0=OP.arith_shift_right)
                # yi = MAGIC - sh  ==  (sh - MAGIC) * -1
                nc.vector.tensor_scalar(out=shv[:], in0=shv[:], scalar1=MAGIC,
                                        scalar2=-1, op0=OP.subtract, op1=OP.mult)
                ry = shv[:].bitcast(f32)
                nc.vector.tensor_tensor(out=rt[:], in0=ry, in1=ry, op=OP.mult)
                nc.vector.scalar_tensor_tensor(out=rt[:], in0=rt[:],
                                               scalar=-0.5, in1=rvv[:],
                                               op0=OP.mult, op1=OP.mult)
                Asb = mp.tile([BLK, NS], f16, tag="Asb")
                nc.vector.scalar_tensor_tensor(out=Asb[:], in0=rt[:], scalar=1.5,
                                               in1=ry, op0=OP.add, op1=OP.mult)
                Bsb = mp.tile([BLK, NS], f16, tag="Bsb")
                nc.vector.tensor_tensor(out=Bsb[:], in0=rmu[:], in1=Asb[:],
                                        op=OP.mult)

                # ============ reg = (mix - mu)*rstd*g (*) df ============
                for k in range(4):
                    nc.vector.tensor_tensor(out=sqf[:, k, :], in0=mix_sb[:, k, :],
                                            in1=Asb[:], op=OP.mult)
                    nc.vector.tensor_tensor(out=sqf[:, k, :], in0=sqf[:, k, :],
                                            in1=Bsb[:], op=OP.subtract)
                    nc.vector.tensor_tensor(out=dff[:, k, :], in0=sqf[:, k, :],
                                            in1=dff[:, k, :], op=OP.mult)
                return dict(reg=dff)

            def back(t, fs):
                reg = fs["reg"]

                # ============ mix MLP ============
                h = hp.tile([BLK, 8, ET], f16, tag="h")
                for m in range(8):
                    acc = pacc.tile([BLK, NS], f32, space="PSUM", tag="a")
                    for k in range(4):
                        nc.tensor.matmul(acc[:], miw1_t[:, k, m * BLK:(m + 1) * BLK],
                                         reg[:, k, :], start=(k == 0), stop=(k == 3))
                    silu_to(h[:, m, :], acc[:], bmi1_t[:, m:m + 1])
                po = pgB.tile([BLK, NS], f32, space="PSUM", tag="b")
                for m in range(8):
                    acc = pacc.tile([BLK, NS], f32, space="PSUM", tag="a")
                    for k in range(8):
                        nc.tensor.matmul(acc[:], miw2_t[:, k, m * BLK:(m + 1) * BLK],
                                         h[:, k, :], start=(k == 0), stop=(k == 7))
                    h2m = sp.tile([BLK, ET], f16, tag="h2m")
                    silu_to(h2m[:], acc[:], bmi2_t[:, m:m + 1])
                    nc.tensor.matmul(po[:], mow_t[:, m, :], h2m[:],
                                     start=(m == 0), stop=(m == 7))
                ot = sp.tile([1, ET], f32, tag="ot")
                nc.scalar.activation(ot[:], po[0:1, :], AF.Identity, bias=bmo_t[:, 0:1],
                                     scale=1.0)
                nc.sync.dma_start(outd[t], ot[:])

            # 2-stage software pipeline: FRONT(t+1) is emitted before
            # BACK(t) so the scheduler always has high-priority PE work
            # during BACK's serial LN/reg chains.
            pend = None
            for t in [tt for _ in range(reps) for tt in range(ntiles)]:
                fs = front(t)
                if fs is None:
                    continue
                if pend is not None:
                    back(*pend)
                pend = (t, fs)
            if pend is not None:
                back(*pend)

    nc.finalize()
    return nc


def _wrap16(v):
    """dma_gather index layout: idx i at [i % 16, i // 16], replicated to
    128 partitions (8 gpsimd cores x 16)."""
    n = v.shape[-1]
    w = np.zeros(v.shape[:-1] + (128, n // 16), np.int16)
    r = v.reshape(v.shape[:-1] + (n // 16, 16))
    for rep in range(8):
        w[..., 16 * rep:16 * (rep + 1), :] = np.swapaxes(r, -1, -2)
    return w


def _host_prep(inputs):
    """Shared (replicated) host-side tensors."""
    f = np.float32
    nodes = np.asarray(inputs["nodes"], f)
    pos = np.asarray(inputs["pos"], f)
    cell = np.asarray(inputs["cell"], f)
    W0 = np.asarray(inputs["W0"], f)
    W1 = np.asarray(inputs["W1"], f)
    W2 = np.asarray(inputs["W2"], f)
    ln_g = np.asarray(inputs["ln_g"], f)

    nodesF = np.zeros((N, BLK), np.float16)
    nodesF[:, :FEAT] = nodes.astype(np.float16)
    bv = np.asarray(inputs["batch_vec"]).astype(np.int64)
    posC = np.zeros((N, 16), f)
    posC[:, :3] = pos
    posC[:, 4:13] = cell.reshape(G, 9)[bv]

    sym = lambda W: 0.5 * (W + W.transpose(1, 0, 2))
    w0f = (sym(W0) / FAN).reshape(L0 * L0, NS)
    w1f = (sym(W1) / (FAN * math.sqrt(3.0))).reshape(L1 * L1, NS)
    w2f = (sym(W2) / (FAN * math.sqrt(5.0))).reshape(L2 * L2, NS)
    h16 = np.float16

    def chunk(w, nch):
        return np.ascontiguousarray(
            w.reshape(nch, BLK, w.shape[1]).transpose(1, 0, 2)).astype(h16)

    miw1 = ln_g[:, None] * np.asarray(inputs["mi_w1"], f)

    def colbias(b, nch):
        b = np.asarray(b, f).reshape(nch, BLK)
        return np.ascontiguousarray(b.T)

    O0, O1, O2 = 0, L0, L0 + 3 * L1
    s0 = np.zeros((BLK, 8 * BLK), h16)
    for c in range(8):
        for p in range(BLK):
            s0[O0 + c * 4 + p // 32, c * BLK + p] = 1.0
    t0 = np.zeros((BLK, BLK), h16)
    for p in range(BLK):
        t0[O0 + p % 32, p] = 1.0
    s1 = np.zeros((BLK, 6 * BLK), h16)
    for c in range(2):
        for i in range(3):
            for p in range(BLK):
                u = c * 8 + p // 16
                s1[O1 + u * 3 + i, (c * 3 + i) * BLK + p] = 1.0
    t1 = np.zeros((BLK, 3 * BLK), h16)
    for i in range(3):
        for p in range(BLK):
            t1[O1 + (p % 16) * 3 + i, i * BLK + p] = 1.0
    # l2 selection matrices, column-padded to 128 per component so FWL
    # stays on; cols 64..127 of each chunk are zero -> psA/psB partitions
    # 64..127 come out zero and the products/mix stay exact.
    s2 = np.zeros((BLK, 5 * BLK), h16)
    t2 = np.zeros((BLK, 5 * BLK), h16)
    for i in range(5):
        for p in range(64):
            s2[O2 + (p // 8) * 5 + i, i * BLK + p] = 1.0
            t2[O2 + (p % 8) * 5 + i, i * BLK + p] = 1.0
    w2full = np.zeros((BLK, NS), h16)
    w2full[:64] = w2f.astype(h16)
    mow = np.asarray(inputs["mo_w"], f).reshape(8, BLK)
    mow128 = np.zeros((BLK, 8, BLK), h16)
    for m in range(8):
        mow128[:, m, 0] = mow[m]
    cn = np.broadcast_to((np.arange(1, NB + 1, dtype=f) / (2.0 * CUT))[None, :],
                         (BLK, NB)).copy()
    return dict(
        nodesF=nodesF, posC=posC,
        w0p=chunk(w0f, 8), w1p=chunk(w1f, 2),
        w2p=w2full,
        dfw1p=chunk(np.asarray(inputs["df_w1"], f), 2),
        dfw2p=chunk(np.asarray(inputs["df_w2"], f), 8),
        miw1p=chunk(miw1, 4),
        miw2p=chunk(np.asarray(inputs["mi_w2"], f), 8),
        mowp=mow128,
        s0d=s0, t0d=t0, s1d=s1, t1d=t1, s2d=s2, t2d=t2,
        bdf1=colbias(inputs["df_b1"], 8), bdf2=colbias(inputs["df_b2"], 4),
        bmi1=colbias(inputs["mi_b1"], 8), bmi2=colbias(inputs["mi_b2"], 8),
        bmo=np.asarray(inputs["mo_b"], f).reshape(1, 1),
        cnd=cn,
    )


def _edge_prep(inputs, core, ntiles):
    """Per-core edge tensors."""
    f = np.float32
    ec = ntiles * ET
    lo = core * EC
    ei = np.asarray(inputs["edge_index"])
    src = ei[0, lo:lo + ec].astype(np.int32)
    dst = ei[1, lo:lo + ec].astype(np.int32)
    shift = np.asarray(inputs["edge_shift"], f)[lo:lo + ec]

    def tile_idx(x):
        return np.ascontiguousarray(x.reshape(ntiles, NBLK, BLK).transpose(0, 2, 1))

    pcidx = np.concatenate([tile_idx(src), tile_idx(dst)], axis=2)

    return dict(
        g16s=_wrap16(src.reshape(ntiles, ET).astype(np.int16)),
        g16d=_wrap16(dst.reshape(ntiles, ET).astype(np.int16)),
        pcidx=pcidx,
        shiftd=np.ascontiguousarray(
            shift.reshape(ntiles, NBLK, BLK, 3).transpose(0, 2, 1, 3)),
    )


def _run(inputs, mode, ntiles, ncores):
    key = (mode, ntiles, 1)
    if key not in _cache:
        _cache[key] = _build(mode, ntiles)
    nc = _cache[key]
    shared = _host_prep(inputs)
    in_maps = []
    for c in range(ncores):
        m = dict(shared)
        m.update(_edge_prep(inputs, c, ntiles))
        in_maps.append(m)

    if mode == "sim":
        from concourse.bass_interp import CoreSim
        outs = []
        for c in range(ncores):
            sim = CoreSim(nc)
            for k, v in in_maps[c].items():
                sim.tensor(k)[:] = v
            sim.simulate()
            outs.append(np.array(sim.tensor("out")).reshape(-1))
        return np.concatenate(outs).reshape(-1, 1)

    from concourse.bass_utils import run_bass_kernel_spmd
    trace = os.environ.get("EXB_TRACE", "0") == "1"
    res = run_bass_kernel_spmd(nc, in_maps, list(range(ncores)), trace=trace)
    out = np.concatenate([res.results[c]["out"].reshape(-1) for c in range(ncores)])
    if trace:
        _run.last_exec_time_ns = res.exec_time_ns
    return out.reshape(-1, 1)


def kernel(**inputs) -> np.ndarray:
    return _run(inputs, os.environ.get("EXB_MODE", "hw"), EC // ET, NCORES).astype(np.float32)


# revision 7
# speedup vs baseline: 1801.2100x; 1.2021x over previous
"""Trainium2 Bass kernel for nn_ExchangeBlock (gnn_message_passing).

Data-parallel over edges: each of the 8 cores processes E/8 = 16384 edges,
node features and weights replicated.  Per 512-edge tile:
  - node features gathered FEATURE-MAJOR via transposing dma_gather (fp16,
    no PE transposes); pos/cell rows via classic indirect DMA
  - radial: tvec, dist (DVE Newton rsqrt), Bessel embedding (range-reduced
    Sin on ACT); embedding transposed to feature-major with DMA XBAR
    transposes (no PE time)
  - fp16 matmuls (free=512 -> 1 cycle/row) for the distance-filter MLP, the
    symmetrized tensor product and the mix MLP.  All stationary operands are
    padded to 128 columns so FWL stays enabled (l2 path zero-padded to the
    full 128 partitions; LN stats via an all-ones [128,128] stationary;
    final mo dot via a column-padded [128,128] stationary).  LayerNorm
    stats run as full-width [128,512] chains on DVE (Newton rsqrt), which
    produces the broadcast A/B tiles directly - no PE broadcast matmuls.
All activations/weights fp16 (quantization ~1e-3), radial + LN-stat math
f32.  PSUM banks: 2 replication A, 2 B/output-row, 3 accumulation
rotation, 1 stats (mu then sq, sequenced).
"""
import os
import sys

sys.path.insert(0, "/opt/trn_rl_repo")

import math
import numpy as np

L0, L1, L2 = 32, 16, 8
NS, NB = 512, 256
CUT = 7.0
N, E, G = 16384, 131072, 16
FEAT = L0 + 3 * L1 + 5 * L2  # 120
NCORES = 8
EC = E // NCORES  # edges per core
BLK = 128
ET = 512  # edges per tile (= one PSUM bank of fp32)
NBLK = ET // BLK
FAN = math.sqrt(float(L0 * L0 + L1 * L1 + L2 * L2))
EMBC = math.sqrt(2.0 / CUT)
MAGIC = 0x5F3759DF

_cache = {}


def _build(mode, ntiles, reps=1, stage=99):
    """Build the Bass program (shared by all cores, SPMD)."""
    import concourse.bacc as bacc
    import concourse.bass as bass
    import concourse.mybir as mybir
    import concourse.tile as tile

    f32 = mybir.dt.float32
    f32r = mybir.dt.float32r
    f16 = mybir.dt.float16
    i32 = mybir.dt.int32
    i16 = mybir.dt.int16
    AF = mybir.ActivationFunctionType
    OP = mybir.AluOpType
    AX = mybir.AxisListType

    nc = bacc.Bacc(None)

    # ---------------- DRAM tensors ----------------
    nodesF = nc.dram_tensor("nodesF", [N, BLK], f16, kind="ExternalInput")
    posC = nc.dram_tensor("posC", [N, 16], f32, kind="ExternalInput")
    g16s = nc.dram_tensor("g16s", [ntiles, BLK, ET // 16], i16, kind="ExternalInput")
    g16d = nc.dram_tensor("g16d", [ntiles, BLK, ET // 16], i16, kind="ExternalInput")
    pcidx = nc.dram_tensor("pcidx", [ntiles, BLK, 8], i32, kind="ExternalInput")
    shiftd = nc.dram_tensor("shiftd", [ntiles, BLK, NBLK, 3], f32, kind="ExternalInput")

    w0p = nc.dram_tensor("w0p", [BLK, 8, NS], f16, kind="ExternalInput")
    w1p = nc.dram_tensor("w1p", [BLK, 2, NS], f16, kind="ExternalInput")
    w2p = nc.dram_tensor("w2p", [BLK, NS], f16, kind="ExternalInput")
    dfw1p = nc.dram_tensor("dfw1p", [BLK, 2, 1024], f16, kind="ExternalInput")
    dfw2p = nc.dram_tensor("dfw2p", [BLK, 8, NS], f16, kind="ExternalInput")
    miw1p = nc.dram_tensor("miw1p", [BLK, 4, 1024], f16, kind="ExternalInput")
    miw2p = nc.dram_tensor("miw2p", [BLK, 8, 1024], f16, kind="ExternalInput")
    mowp = nc.dram_tensor("mowp", [BLK, 8, BLK], f16, kind="ExternalInput")
    s0d = nc.dram_tensor("s0d", [BLK, 8 * BLK], f16, kind="ExternalInput")
    t0d = nc.dram_tensor("t0d", [BLK, BLK], f16, kind="ExternalInput")
    s1d = nc.dram_tensor("s1d", [BLK, 6 * BLK], f16, kind="ExternalInput")
    t1d = nc.dram_tensor("t1d", [BLK, 3 * BLK], f16, kind="ExternalInput")
    s2d = nc.dram_tensor("s2d", [BLK, 5 * BLK], f16, kind="ExternalInput")
    t2d = nc.dram_tensor("t2d", [BLK, 5 * BLK], f16, kind="ExternalInput")
    bdf1 = nc.dram_tensor("bdf1", [BLK, 8], f32, kind="ExternalInput")
    bdf2 = nc.dram_tensor("bdf2", [BLK, 4], f32, kind="ExternalInput")
    bmi1 = nc.dram_tensor("bmi1", [BLK, 8], f32, kind="ExternalInput")
    bmi2 = nc.dram_tensor("bmi2", [BLK, 8], f32, kind="ExternalInput")
    bmo = nc.dram_tensor("bmo", [1, 1], f32, kind="ExternalInput")
    cnd = nc.dram_tensor("cnd", [BLK, NB], f32, kind="ExternalInput")
    identd = nc.dram_tensor("identd", [BLK, BLK], f16, kind="ExternalInput")

    outd = nc.dram_tensor("out", [ntiles, 1, ET], f32, kind="ExternalOutput")

    TWO_PI = 2.0 * math.pi
    sin_bias = -math.pi if mode == "sim" else 0.0
    dscale_c = -EMBC if mode == "sim" else EMBC

    with tile.TileContext(nc) as tc:
        with (
            tc.tile_pool(name="const", bufs=1) as cp,
            tc.tile_pool(name="gat", bufs=3) as gp,
            tc.tile_pool(name="rad", bufs=3) as rp,
            tc.tile_pool(name="emb", bufs=2) as ep,
            tc.tile_pool(name="trx", bufs=3) as xp,
            tc.tile_pool(name="tpp", bufs=2) as tp,
            tc.tile_pool(name="mid", bufs=2) as mp,
            tc.tile_pool(name="row", bufs=1) as wp,
            tc.tile_pool(name="h1s", bufs=2) as hp1,
            tc.tile_pool(name="hs", bufs=2) as hp,
            tc.tile_pool(name="sml", bufs=2) as sp,
            tc.tile_pool(name="psA", bufs=2, space="PSUM") as pgA,
            tc.tile_pool(name="psB", bufs=2, space="PSUM") as pgB,
            tc.tile_pool(name="pac", bufs=3, space="PSUM") as pacc,
            tc.tile_pool(name="pst", bufs=1, space="PSUM") as pst,
        ):
            # ---------------- constants ----------------
            w0_t = cp.tile([BLK, 8, NS], f16)
            nc.sync.dma_start(w0_t[:], w0p[:])
            w1_t = cp.tile([BLK, 2, NS], f16)
            nc.sync.dma_start(w1_t[:], w1p[:])
            w2_t = cp.tile([BLK, NS], f16)
            nc.sync.dma_start(w2_t[:], w2p[:])
            dfw1_t = cp.tile([BLK, 2, 1024], f16)
            nc.sync.dma_start(dfw1_t[:], dfw1p[:])
            dfw2_t = cp.tile([BLK, 8, NS], f16)
            nc.sync.dma_start(dfw2_t[:], dfw2p[:])
            miw1_t = cp.tile([BLK, 4, 1024], f16)
            nc.sync.dma_start(miw1_t[:], miw1p[:])
            miw2_t = cp.tile([BLK, 8, 1024], f16)
            nc.sync.dma_start(miw2_t[:], miw2p[:])
            mow_t = cp.tile([BLK, 8, BLK], f16)
            nc.sync.dma_start(mow_t[:], mowp[:])
            s0_t = cp.tile([BLK, 8 * BLK], f16)
            nc.sync.dma_start(s0_t[:], s0d[:])
            t0_t = cp.tile([BLK, BLK], f16)
            nc.sync.dma_start(t0_t[:], t0d[:])
            s1_t = cp.tile([BLK, 6 * BLK], f16)
            nc.sync.dma_start(s1_t[:], s1d[:])
            t1_t = cp.tile([BLK, 3 * BLK], f16)
            nc.sync.dma_start(t1_t[:], t1d[:])
            s2_t = cp.tile([BLK, 5 * BLK], f16)
            nc.sync.dma_start(s2_t[:], s2d[:])
            t2_t = cp.tile([BLK, 5 * BLK], f16)
            nc.sync.dma_start(t2_t[:], t2d[:])
            bdf1_t = cp.tile([BLK, 8], f32)
            nc.sync.dma_start(bdf1_t[:], bdf1[:])
            bdf2_t = cp.tile([BLK, 4], f32)
            nc.sync.dma_start(bdf2_t[:], bdf2[:])
            bmi1_t = cp.tile([BLK, 8], f32)
            nc.sync.dma_start(bmi1_t[:], bmi1[:])
            bmi2_t = cp.tile([BLK, 8], f32)
            nc.sync.dma_start(bmi2_t[:], bmi2[:])
            bmo_t = cp.tile([1, 1], f32)
            nc.sync.dma_start(bmo_t[:], bmo[:])
            cn_t = cp.tile([BLK, NB], f32)
            nc.sync.dma_start(cn_t[:], cnd[:])
            ones128_t = cp.tile([BLK, BLK], f16)
            nc.gpsimd.memset(ones128_t[:], 1.0)
            id_t = cp.tile([BLK, BLK], f16)
            nc.sync.dma_start(id_t[:], identd[:])
            magic_t = cp.tile([BLK, NBLK], i32)
            nc.gpsimd.memset(magic_t[:], MAGIC)
            sinb_t = cp.tile([BLK, 1], f32)
            nc.gpsimd.memset(sinb_t[:], sin_bias)

            def silu_to(dst, ps, bias_ap):
                if mode == "sim":
                    sg = sp.tile([BLK, ET], f16, tag="sg")
                    nc.scalar.activation(sg[:], ps, AF.Sigmoid, bias=bias_ap, scale=1.0)
                    pre = sp.tile([BLK, ET], f16, tag="pre")
                    nc.vector.tensor_scalar(out=pre[:], in0=ps, scalar1=bias_ap,
                                            scalar2=None, op0=OP.add)
                    nc.vector.tensor_tensor(out=dst, in0=sg[:], in1=pre[:], op=OP.mult)
                else:
                    nc.scalar.activation(dst, ps, AF.Silu, bias=bias_ap, scale=1.0)

            def front(t):
                # ============ index DMAs ============
                if stage < 0:
                    return None
                gs16 = gp.tile([BLK, ET // 16], i16, tag="gs16")
                gd16 = gp.tile([BLK, ET // 16], i16, tag="gd16")
                nc.sync.dma_start(gs16[:], g16s[t])
                nc.sync.dma_start(gd16[:], g16d[t])
                pci = gp.tile([BLK, 8], i32, tag="pci")
                nc.sync.dma_start(pci[:], pcidx[t])
                shf = gp.tile([BLK, NBLK, 3], f32, tag="shf")
                nc.sync.dma_start(shf[:], shiftd[t])

                # ============ gathers ============
                xsT = xp.tile([BLK, 1, ET], f16, tag="xsT")
                nc.gpsimd.dma_gather(
                    out_ap=xsT[:], in_ap=nodesF[:], idxs_ap=gs16[:],
                    num_idxs=ET, num_idxs_reg=ET, elem_size=BLK, transpose=True)
                xdT = xp.tile([BLK, 1, ET], f16, tag="xdT")
                nc.gpsimd.dma_gather(
                    out_ap=xdT[:], in_ap=nodesF[:], idxs_ap=gd16[:],
                    num_idxs=ET, num_idxs_reg=ET, elem_size=BLK, transpose=True)
                gsc = gp.tile([BLK, NBLK, 16], f32, tag="gsc")
                gdc = gp.tile([BLK, NBLK, 16], f32, tag="gdc")
                for b in range(NBLK):
                    nc.gpsimd.indirect_dma_start(
                        out=gsc[:, b, :], out_offset=None, in_=posC[:],
                        in_offset=bass.IndirectOffsetOnAxis(ap=pci[:, b:b + 1], axis=0))
                    nc.gpsimd.indirect_dma_start(
                        out=gdc[:, b, :], out_offset=None, in_=posC[:],
                        in_offset=bass.IndirectOffsetOnAxis(ap=pci[:, 4 + b:5 + b], axis=0))

                if stage < 1:
                    ot0 = sp.tile([1, ET], f32, tag="ot")
                    nc.vector.tensor_copy(ot0[:], xsT[0:1, 0, :])
                    nc.sync.dma_start(outd[t], ot0[:])
                    return None
                # ============ radial (f32) ============
                prod = rp.tile([BLK, NBLK, 3, 3], f32, tag="prod")
                nc.vector.tensor_tensor(
                    out=prod[:],
                    in0=gsc[:, :, 4:13].rearrange("p b (i j) -> p b j i", i=3, j=3),
                    in1=shf[:].unsqueeze(2).to_broadcast([BLK, NBLK, 3, 3]),
                    op=OP.mult)
                tvec = rp.tile([BLK, NBLK, 3], f32, tag="tvec")
                nc.vector.tensor_reduce(out=tvec[:], in_=prod[:], axis=AX.X, op=OP.add)
                rv = rp.tile([BLK, NBLK, 3], f32, tag="rv")
                nc.vector.tensor_tensor(out=rv[:], in0=gdc[:, :, 0:3],
                                        in1=gsc[:, :, 0:3], op=OP.subtract)
                nc.vector.tensor_tensor(out=rv[:], in0=rv[:], in1=tvec[:], op=OP.add)
                sq = rp.tile([BLK, NBLK, 3], f32, tag="sq")
                nc.vector.tensor_tensor(out=sq[:], in0=rv[:], in1=rv[:], op=OP.mult)
                d2 = rp.tile([BLK, NBLK], f32, tag="d2")
                nc.vector.tensor_reduce(out=d2[:], in_=sq[:], axis=AX.X, op=OP.add)
                nc.vector.tensor_scalar(out=d2[:], in0=d2[:], scalar1=1e-24,
                                        scalar2=None, op0=OP.max)
                # Newton rsqrt (2 iters)
                sh = rp.tile([BLK, NBLK], i32, tag="sh")
                nc.vector.tensor_scalar(out=sh[:], in0=d2[:].bitcast(i32), scalar1=1,
                                        scalar2=None, op0=OP.arith_shift_right)
                yi = rp.tile([BLK, NBLK], i32, tag="yi")
                nc.vector.tensor_tensor(out=yi[:], in0=magic_t[:], in1=sh[:],
                                        op=OP.subtract)
                y = yi[:].bitcast(f32)
                d2h = rp.tile([BLK, NBLK], f32, tag="d2h")
                nc.vector.tensor_scalar(out=d2h[:], in0=d2[:], scalar1=0.5,
                                        scalar2=None, op0=OP.mult)
                tmp = rp.tile([BLK, NBLK], f32, tag="tmp")
                for _ in range(2):
                    nc.vector.tensor_tensor(out=tmp[:], in0=y, in1=y, op=OP.mult)
                    nc.vector.tensor_tensor(out=tmp[:], in0=tmp[:], in1=d2h[:], op=OP.mult)
                    nc.vector.tensor_scalar(out=tmp[:], in0=tmp[:], scalar1=-1.0,
                                            scalar2=1.5, op0=OP.mult, op1=OP.add)
                    nc.vector.tensor_tensor(out=yi[:].bitcast(f32), in0=y, in1=tmp[:],
                                            op=OP.mult)
                dist = rp.tile([BLK, NBLK], f32, tag="dist")
                nc.vector.tensor_tensor(out=dist[:], in0=d2[:], in1=y, op=OP.mult)
                nc.vector.tensor_scalar(out=dist[:], in0=dist[:], scalar1=1e-6,
                                        scalar2=None, op0=OP.add)
                # r = 1/(dist+1e-6), one NR step from seed y
                nc.vector.tensor_tensor(out=tmp[:], in0=dist[:], in1=y, op=OP.mult)
                nc.vector.tensor_scalar(out=tmp[:], in0=tmp[:], scalar1=-1.0,
                                        scalar2=2.0, op0=OP.mult, op1=OP.add)
                r_ = rp.tile([BLK, NBLK], f32, tag="r_")
                nc.vector.tensor_tensor(out=r_[:], in0=y, in1=tmp[:], op=OP.mult)
                dsc = rp.tile([BLK, NBLK], f32, tag="dsc")
                nc.vector.tensor_scalar(out=dsc[:], in0=dist[:], scalar1=dscale_c,
                                        scalar2=None, op0=OP.mult)

                # ============ embedding (edge-major) ============
                u = ep.tile([BLK, NBLK, NB], f32, tag="u")
                nc.vector.tensor_tensor(
                    out=u[:], in0=r_[:].unsqueeze(2).to_broadcast([BLK, NBLK, NB]),
                    in1=cn_t[:].unsqueeze(1).to_broadcast([BLK, NBLK, NB]), op=OP.mult)
                icv = ep.tile([BLK, NBLK, NB], i16, tag="icv")
                nc.vector.tensor_copy(icv[:], u[:])
                nc.vector.tensor_tensor(out=u[:], in0=u[:], in1=icv[:], op=OP.subtract)
                sinv = ep.tile([BLK, NBLK * NB], f16, tag="sinv")
                sinv_v = sinv[:].rearrange("p (b n) -> p b n", b=NBLK)
                nc.scalar.activation(sinv_v, u[:], AF.Sin, bias=sinb_t[:, 0:1],
                                     scale=TWO_PI)
                for b in range(NBLK):
                    nc.scalar.activation(sinv_v[:, b, :], sinv_v[:, b, :], AF.Copy,
                                         scale=dsc[:, b:b + 1])

                if stage < 2:
                    ot0 = sp.tile([1, ET], f32, tag="ot")
                    nc.vector.tensor_copy(ot0[:], sinv[0:1, 0:ET])
                    nc.sync.dma_start(outd[t], ot0[:])
                    return None
                if stage < 3:
                    ot0 = sp.tile([1, ET], f32, tag="ot")
                    nc.vector.tensor_copy(ot0[:], embT[0:1, 0, :])
                    nc.sync.dma_start(outd[t], ot0[:])
                    return None
                # ============ TP replication + products ============
                prod16 = tp.tile([BLK, 11, ET], f16, tag="prod16")
                psB0 = pgB.tile([BLK, NS], f32, space="PSUM", tag="b")
                nc.tensor.matmul(psB0[:], t0_t[:], xdT[:, 0, :], start=True, stop=True)
                xd0t = tp.tile([BLK, ET], f16, tag="xd0t")
                nc.scalar.copy(xd0t[:], psB0[:])
                for c in range(8):
                    psA = pgA.tile([BLK, NS], f32, space="PSUM", tag="a")
                    nc.tensor.matmul(psA[:], s0_t[:, c * BLK:(c + 1) * BLK],
                                     xsT[:, 0, :], start=True, stop=True)
                    nc.vector.tensor_tensor(out=prod16[:, c, :], in0=psA[:],
                                            in1=xd0t[:], op=OP.mult)
                for i in range(3):
                    psB = pgB.tile([BLK, NS], f32, space="PSUM", tag="b")
                    nc.tensor.matmul(psB[:], t1_t[:, i * BLK:(i + 1) * BLK],
                                     xdT[:, 0, :], start=True, stop=True)
                    bsb = tp.tile([BLK, ET], f16, tag="bsb")
                    nc.scalar.copy(bsb[:], psB[:])
                    for c in range(2):
                        psA = pgA.tile([BLK, NS], f32, space="PSUM", tag="a")
                        nc.tensor.matmul(psA[:], s1_t[:, (c * 3 + i) * BLK:(c * 3 + i + 1) * BLK],
                                         xsT[:, 0, :], start=True, stop=True)
                        if i == 0:
                            nc.vector.tensor_tensor(out=prod16[:, 8 + c, :], in0=psA[:],
                                                    in1=bsb[:], op=OP.mult)
                        else:
                            tmq = tp.tile([BLK, ET], f16, tag="tmq")
                            nc.vector.tensor_tensor(out=tmq[:], in0=psA[:], in1=bsb[:],
                                                    op=OP.mult)
                            nc.vector.tensor_tensor(out=prod16[:, 8 + c, :],
                                                    in0=prod16[:, 8 + c, :],
                                                    in1=tmq[:], op=OP.add)
                for i in range(5):
                    psB = pgB.tile([BLK, NS], f32, space="PSUM", tag="b")
                    nc.tensor.matmul(psB[:], t2_t[:, i * BLK:(i + 1) * BLK],
                                     xdT[:, 0, :], start=True, stop=True)
                    bsb2 = tp.tile([BLK, ET], f16, tag="bsb2")
                    nc.scalar.copy(bsb2[:], psB[:])
                    psA = pgA.tile([BLK, NS], f32, space="PSUM", tag="a")
                    nc.tensor.matmul(psA[:], s2_t[:, i * BLK:(i + 1) * BLK],
                                     xsT[:, 0, :], start=True, stop=True)
                    if i == 0:
                        nc.vector.tensor_tensor(out=prod16[:, 10, :], in0=psA[:],
                                                in1=bsb2[:], op=OP.mult)
                    else:
                        tmq2 = tp.tile([BLK, ET], f16, tag="tmq")
                        nc.vector.tensor_tensor(out=tmq2[:], in0=psA[:],
                                                in1=bsb2[:], op=OP.mult)
                        nc.vector.tensor_tensor(out=prod16[:, 10, :],
                                                in0=prod16[:, 10, :],
                                                in1=tmq2[:], op=OP.add)

                if stage < 4:
                    ot0 = sp.tile([1, ET], f32, tag="ot")
                    nc.vector.tensor_copy(ot0[:], prod16[0:1, 0, :])
                    nc.sync.dma_start(outd[t], ot0[:])
                    return None
                # ===== mix einsum (c-order in m passes) + LN mu stats =====
                mix_sb = mp.tile([BLK, 4, NS], f16, tag="mix_sb")
                sqf = mp.tile([BLK, 4, NS], f16, tag="sqf")
                mu_ps = pst.tile([BLK, NS], f32, space="PSUM", tag="st")

                def mix_chunk(acc, c, m, start):
                    if c < 8:
                        nc.tensor.matmul(acc[:], w0_t[:, c, m * BLK:(m + 1) * BLK],
                                         prod16[:, c, :], start=start, stop=False)
                    elif c < 10:
                        nc.tensor.matmul(acc[:], w1_t[:, c - 8, m * BLK:(m + 1) * BLK],
                                         prod16[:, c, :], start=start, stop=False)
                    else:
                        nc.tensor.matmul(acc[:], w2_t[:, m * BLK:(m + 1) * BLK],
                                         prod16[:, 10, :], start=start, stop=True)

                for m in range(4):
                    acc = pacc.tile([BLK, NS], f32, space="PSUM", tag="a")
                    for c in range(11):
                        mix_chunk(acc, c, m, start=(c == 0))
                    nc.scalar.copy(mix_sb[:, m, :], acc[:])
                    nc.vector.tensor_tensor(out=sqf[:, m, :], in0=mix_sb[:, m, :],
                                            in1=mix_sb[:, m, :], op=OP.mult)
                    nc.tensor.matmul(mu_ps[:], ones128_t[:], mix_sb[:, m, :],
                                     start=(m == 0), stop=(m == 3))

                if stage < 5:
                    ot0 = sp.tile([1, ET], f32, tag="ot")
                    nc.vector.tensor_copy(ot0[:], mix_sb[0:1, 0, :])
                    nc.sync.dma_start(outd[t], ot0[:])
                    return None

                # mu consumed early so the stats bank can be reused for sq
                rmu = wp.tile([BLK, NS], f32, tag="rmu")
                nc.vector.tensor_scalar(out=rmu[:], in0=mu_ps[:], scalar1=1.0 / NS,
                                        scalar2=None, op0=OP.mult)
                s2_ps = pst.tile([BLK, NS], f32, space="PSUM", tag="st")

                # embT via PE transposes (fp16, 1c/row)
                embT = xp.tile([BLK, 2, ET], f16, tag="embT")
                sinb = sinv[:].rearrange("p (b n) -> p b n", b=NBLK)
                for b in range(NBLK):
                    for k in range(2):
                        pt = pgA.tile([BLK, NS], f32, space="PSUM", tag="a")
                        ptv = pt[:].bitcast(f16)[:, 0:BLK]
                        nc.tensor.transpose(ptv,
                                            sinb[:, b, k * BLK:(k + 1) * BLK],
                                            id_t[:])
                        nc.scalar.copy(embT[:, k, b * BLK:(b + 1) * BLK], ptv)

                # ============ df MLP layer 1 (+ sq stats interleaved) ======
                h1c = hp1.tile([BLK, 8, ET], f16, tag="h1c")
                for m in range(8):
                    acc = pacc.tile([BLK, NS], f32, space="PSUM", tag="a")
                    for k in range(2):
                        nc.tensor.matmul(acc[:], dfw1_t[:, k, m * BLK:(m + 1) * BLK],
                                         embT[:, k, :], start=(k == 0), stop=(k == 1))
                    silu_to(h1c[:, m, :], acc[:], bdf1_t[:, m:m + 1])
                    if m < 4:
                        nc.tensor.matmul(s2_ps[:], ones128_t[:], sqf[:, m, :],
                                         start=(m == 0), stop=(m == 3))
                # ============ df MLP layer 2 ============
                dff = mp.tile([BLK, 4, NS], f16, tag="dff")
                for m in range(4):
                    acc = pacc.tile([BLK, NS], f32, space="PSUM", tag="a")
                    for k in range(8):
                        nc.tensor.matmul(acc[:], dfw2_t[:, k, m * BLK:(m + 1) * BLK],
                                         h1c[:, k, :], start=(k == 0), stop=(k == 7))
                    nc.scalar.activation(dff[:, m, :], acc[:], AF.Identity,
                                         bias=bdf2_t[:, m:m + 1], scale=1.0)

                # ============ LN chain, full [128, NS] width (f32) ========
                rt = wp.tile([BLK, NS], f32, tag="rt")
                nc.vector.tensor_tensor(out=rt[:], in0=rmu[:], in1=rmu[:], op=OP.mult)
                rvv = wp.tile([BLK, NS], f32, tag="rvv")
                nc.vector.scalar_tensor_tensor(out=rvv[:], in0=s2_ps[:],
                                               scalar=1.0 / NS, in1=rt[:],
                                               op0=OP.mult, op1=OP.subtract)
                nc.vector.tensor_scalar(out=rvv[:], in0=rvv[:], scalar1=1e-5,
                                        scalar2=None, op0=OP.add)
                shv = wp.tile([BLK, NS], i32, tag="shv")
                nc.vector.tensor_scalar(out=shv[:], in0=rvv[:].bitcast(i32), scalar1=1,
                                        scalar2=None, op0=OP.arith_shift_right)
                # yi = MAGIC - sh  ==  (sh - MAGIC) * -1
                nc.vector.tensor_scalar(out=shv[:], in0=shv[:], scalar1=MAGIC,
                                        scalar2=-1, op0=OP.subtract, op1=OP.mult)
                ry = shv[:].bitcast(f32)
                nc.vector.tensor_tensor(out=rt[:], in0=ry, in1=ry, op=OP.mult)
                nc.vector.scalar_tensor_tensor(out=rt[:], in0=rt[:],
                                               scalar=-0.5, in1=rvv[:],
                                               op0=OP.mult, op1=OP.mult)
                Asb = mp.tile([BLK, NS], f16, tag="Asb")
                nc.vector.scalar_tensor_tensor(out=Asb[:], in0=rt[:], scalar=1.5,
                                               in1=ry, op0=OP.add, op1=OP.mult)
                Bsb = mp.tile([BLK, NS], f16, tag="Bsb")
                nc.vector.tensor_tensor(out=Bsb[:], in0=rmu[:], in1=Asb[:],
                                        op=OP.mult)

                # ============ reg = (mix - mu)*rstd*g (*) df ============
                for k in range(4):
                    nc.vector.tensor_tensor(out=sqf[:, k, :], in0=mix_sb[:, k, :],
                                            in1=Asb[:], op=OP.mult)
                    nc.vector.tensor_tensor(out=sqf[:, k, :], in0=sqf[:, k, :],
                                            in1=Bsb[:], op=OP.subtract)
                    nc.vector.tensor_tensor(out=dff[:, k, :], in0=sqf[:, k, :],
                                            in1=dff[:, k, :], op=OP.mult)
                return dict(reg=dff)

            def back(t, fs):
                reg = fs["reg"]

                # ============ mix MLP ============
                h = hp.tile([BLK, 8, ET], f16, tag="h")
                for m in range(8):
                    acc = pacc.tile([BLK, NS], f32, space="PSUM", tag="a")
                    for k in range(4):
                        nc.tensor.matmul(acc[:], miw1_t[:, k, m * BLK:(m + 1) * BLK],
                                         reg[:, k, :], start=(k == 0), stop=(k == 3))
                    silu_to(h[:, m, :], acc[:], bmi1_t[:, m:m + 1])
                po = pgB.tile([BLK, NS], f32, space="PSUM", tag="b")
                for m in range(8):
                    acc = pacc.tile([BLK, NS], f32, space="PSUM", tag="a")
                    for k in range(8):
                        nc.tensor.matmul(acc[:], miw2_t[:, k, m * BLK:(m + 1) * BLK],
                                         h[:, k, :], start=(k == 0), stop=(k == 7))
                    h2m = sp.tile([BLK, ET], f16, tag="h2m")
                    silu_to(h2m[:], acc[:], bmi2_t[:, m:m + 1])
                    nc.tensor.matmul(po[:], mow_t[:, m, :], h2m[:],
                                     start=(m == 0), stop=(m == 7))
                ot = sp.tile([1, ET], f32, tag="ot")
                nc.scalar.activation(ot[:], po[0:1, :], AF.Identity, bias=bmo_t[:, 0:1],
                                     scale=1.0)
                nc.sync.dma_start(outd[t], ot[:])

            # 2-stage software pipeline: FRONT(t+1) is emitted before
            # BACK(t) so the scheduler always has high-priority PE work
            # during BACK's serial LN/reg chains.
            pend = None
            for t in [tt for _ in range(reps) for tt in range(ntiles)]:
                fs = front(t)
                if fs is None:
                    continue
                if pend is not None:
                    back(*pend)
                pend = (t, fs)
            if pend is not None:
                back(*pend)

    nc.finalize()
    return nc


def _wrap16(v):
    """dma_gather index layout: idx i at [i % 16, i // 16], replicated to
    128 partitions (8 gpsimd cores x 16)."""
    n = v.shape[-1]
    w = np.zeros(v.shape[:-1] + (128, n // 16), np.int16)
    r = v.reshape(v.shape[:-1] + (n // 16, 16))
    for rep in range(8):
        w[..., 16 * rep:16 * (rep + 1), :] = np.swapaxes(r, -1, -2)
    return w


def _host_prep(inputs):
    """Shared (replicated) host-side tensors."""
    f = np.float32
    nodes = np.asarray(inputs["nodes"], f)
    pos = np.asarray(inputs["pos"], f)
    cell = np.asarray(inputs["cell"], f)
    W0 = np.asarray(inputs["W0"], f)
    W1 = np.asarray(inputs["W1"], f)
    W2 = np.asarray(inputs["W2"], f)
    ln_g = np.asarray(inputs["ln_g"], f)

    nodesF = np.zeros((N, BLK), np.float16)
    nodesF[:, :FEAT] = nodes.astype(np.float16)
    bv = np.asarray(inputs["batch_vec"]).astype(np.int64)
    posC = np.zeros((N, 16), f)
    posC[:, :3] = pos
    posC[:, 4:13] = cell.reshape(G, 9)[bv]

    sym = lambda W: 0.5 * (W + W.transpose(1, 0, 2))
    w0f = (sym(W0) / FAN).reshape(L0 * L0, NS)
    w1f = (sym(W1) / (FAN * math.sqrt(3.0))).reshape(L1 * L1, NS)
    w2f = (sym(W2) / (FAN * math.sqrt(5.0))).reshape(L2 * L2, NS)
    h16 = np.float16

    def chunk(w, nch):
        return np.ascontiguousarray(
            w.reshape(nch, BLK, w.shape[1]).transpose(1, 0, 2)).astype(h16)

    miw1 = ln_g[:, None] * np.asarray(inputs["mi_w1"], f)

    def colbias(b, nch):
        b = np.asarray(b, f).reshape(nch, BLK)
        return np.ascontiguousarray(b.T)

    O0, O1, O2 = 0, L0, L0 + 3 * L1
    s0 = np.zeros((BLK, 8 * BLK), h16)
    for c in range(8):
        for p in range(BLK):
            s0[O0 + c * 4 + p // 32, c * BLK + p] = 1.0
    t0 = np.zeros((BLK, BLK), h16)
    for p in range(BLK):
        t0[O0 + p % 32, p] = 1.0
    s1 = np.zeros((BLK, 6 * BLK), h16)
    for c in range(2):
        for i in range(3):
            for p in range(BLK):
                u = c * 8 + p // 16
                s1[O1 + u * 3 + i, (c * 3 + i) * BLK + p] = 1.0
    t1 = np.zeros((BLK, 3 * BLK), h16)
    for i in range(3):
        for p in range(BLK):
            t1[O1 + (p % 16) * 3 + i, i * BLK + p] = 1.0
    # l2 selection matrices, column-padded to 128 per component so FWL
    # stays on; cols 64..127 of each chunk are zero -> psA/psB partitions
    # 64..127 come out zero and the products/mix stay exact.
    s2 = np.zeros((BLK, 5 * BLK), h16)
    t2 = np.zeros((BLK, 5 * BLK), h16)
    for i in range(5):
        for p in range(64):
            s2[O2 + (p // 8) * 5 + i, i * BLK + p] = 1.0
            t2[O2 + (p % 8) * 5 + i, i * BLK + p] = 1.0
    w2full = np.zeros((BLK, NS), h16)
    w2full[:64] = w2f.astype(h16)
    mow = np.asarray(inputs["mo_w"], f).reshape(8, BLK)
    mow128 = np.zeros((BLK, 8, BLK), h16)
    for m in range(8):
        mow128[:, m, 0] = mow[m]
    cn = np.broadcast_to((np.arange(1, NB + 1, dtype=f) / (2.0 * CUT))[None, :],
                         (BLK, NB)).copy()
    return dict(
        nodesF=nodesF, posC=posC,
        w0p=chunk(w0f, 8), w1p=chunk(w1f, 2),
        w2p=w2full,
        dfw1p=chunk(np.asarray(inputs["df_w1"], f), 2),
        dfw2p=chunk(np.asarray(inputs["df_w2"], f), 8),
        miw1p=chunk(miw1, 4),
        miw2p=chunk(np.asarray(inputs["mi_w2"], f), 8),
        mowp=mow128,
        s0d=s0, t0d=t0, s1d=s1, t1d=t1, s2d=s2, t2d=t2,
        bdf1=colbias(inputs["df_b1"], 8), bdf2=colbias(inputs["df_b2"], 4),
        bmi1=colbias(inputs["mi_b1"], 8), bmi2=colbias(inputs["mi_b2"], 8),
        bmo=np.asarray(inputs["mo_b"], f).reshape(1, 1),
        cnd=cn,
        identd=np.eye(BLK, dtype=h16),
    )


def _edge_prep(inputs, core, ntiles):
    """Per-core edge tensors."""
    f = np.float32
    ec = ntiles * ET
    lo = core * EC
    ei = np.asarray(inputs["edge_index"])
    src = ei[0, lo:lo + ec].astype(np.int32)
    dst = ei[1, lo:lo + ec].astype(np.int32)
    shift = np.asarray(inputs["edge_shift"], f)[lo:lo + ec]

    def tile_idx(x):
        return np.ascontiguousarray(x.reshape(ntiles, NBLK, BLK).transpose(0, 2, 1))

    pcidx = np.concatenate([tile_idx(src), tile_idx(dst)], axis=2)

    return dict(
        g16s=_wrap16(src.reshape(ntiles, ET).astype(np.int16)),
        g16d=_wrap16(dst.reshape(ntiles, ET).astype(np.int16)),
        pcidx=pcidx,
        shiftd=np.ascontiguousarray(
            shift.reshape(ntiles, NBLK, BLK, 3).transpose(0, 2, 1, 3)),
    )


def _run(inputs, mode, ntiles, ncores):
    key = (mode, ntiles, 1)
    if key not in _cache:
        _cache[key] = _build(mode, ntiles)
    nc = _cache[key]
    shared = _host_prep(inputs)
    in_maps = []
    for c in range(ncores):
        m = dict(shared)
        m.update(_edge_prep(inputs, c, ntiles))
        in_maps.append(m)

    if mode == "sim":
        from concourse.bass_interp import CoreSim
        outs = []
        for c in range(ncores):
            sim = CoreSim(nc)
            for k, v in in_maps[c].items():
                sim.tensor(k)[:] = v
            sim.simulate()
            outs.append(np.array(sim.tensor("out")).reshape(-1))
        return np.concatenate(outs).reshape(-1, 1)

    from concourse.bass_utils import run_bass_kernel_spmd
    trace = os.environ.get("EXB_TRACE", "0") == "1"
    res = run_bass_kernel_spmd(nc, in_maps, list(range(ncores)), trace=trace)
    out = np.concatenate([res.results[c]["out"].reshape(-1) for c in range(ncores)])
    if trace:
        _run.last_exec_time_ns = res.exec_time_ns
    return out.reshape(-1, 1)


def kernel(**inputs) -> np.ndarray:
    return _run(inputs, os.environ.get("EXB_MODE", "hw"), EC // ET, NCORES).astype(np.float32)


# revision 10
# speedup vs baseline: 1953.3880x; 1.0845x over previous
"""Trainium2 Bass kernel for nn_ExchangeBlock (gnn_message_passing).

Data-parallel over edges: each of the 8 cores processes E/8 = 16384 edges,
node features and weights replicated.  Per 512-edge tile:
  - node features gathered FEATURE-MAJOR via transposing dma_gather (fp16,
    no PE transposes); pos/cell rows via classic indirect DMA
  - radial: tvec, dist (DVE Newton rsqrt), Bessel embedding (range-reduced
    Sin on ACT); embedding transposed to feature-major with DMA XBAR
    transposes (no PE time)
  - fp16 matmuls (free=512 -> 1 cycle/row) for the distance-filter MLP, the
    symmetrized tensor product and the mix MLP.  All stationary operands are
    padded to 128 columns so FWL stays enabled (l2 path zero-padded to the
    full 128 partitions; LN stats via an all-ones [128,128] stationary;
    final mo dot via a column-padded [128,128] stationary).  LayerNorm
    stats run as full-width [128,512] chains on DVE (Newton rsqrt), which
    produces the broadcast A/B tiles directly - no PE broadcast matmuls.
All activations/weights fp16 (quantization ~1e-3), radial + LN-stat math
f32.  PSUM banks: 2 replication A, 2 B/output-row, 3 accumulation
rotation, 1 stats (mu then sq, sequenced).
"""
import os
import sys

sys.path.insert(0, "/opt/trn_rl_repo")

import math
import numpy as np

L0, L1, L2 = 32, 16, 8
NS, NB = 512, 256
CUT = 7.0
N, E, G = 16384, 131072, 16
FEAT = L0 + 3 * L1 + 5 * L2  # 120
NCORES = 8
EC = E // NCORES  # edges per core
BLK = 128
ET = 512  # edges per tile (= one PSUM bank of fp32)
NBLK = ET // BLK
FAN = math.sqrt(float(L0 * L0 + L1 * L1 + L2 * L2))
EMBC = math.sqrt(2.0 / CUT)
MAGIC = 0x5F3759DF

_cache = {}


def _build(mode, ntiles, reps=1, stage=99):
    """Build the Bass program (shared by all cores, SPMD)."""
    import concourse.bacc as bacc
    import concourse.bass as bass
    import concourse.mybir as mybir
    import concourse.tile as tile

    f32 = mybir.dt.float32
    f32r = mybir.dt.float32r
    f16 = mybir.dt.float16
    i32 = mybir.dt.int32
    i16 = mybir.dt.int16
    AF = mybir.ActivationFunctionType
    OP = mybir.AluOpType
    AX = mybir.AxisListType

    nc = bacc.Bacc(None)

    # ---------------- DRAM tensors ----------------
    nodesF = nc.dram_tensor("nodesF", [N, BLK], f16, kind="ExternalInput")
    posC = nc.dram_tensor("posC", [N, 16], f32, kind="ExternalInput")
    g16s = nc.dram_tensor("g16s", [ntiles, BLK, ET // 16], i16, kind="ExternalInput")
    g16d = nc.dram_tensor("g16d", [ntiles, BLK, ET // 16], i16, kind="ExternalInput")
    pcidx = nc.dram_tensor("pcidx", [ntiles, BLK, 8], i32, kind="ExternalInput")
    shiftd = nc.dram_tensor("shiftd", [ntiles, BLK, NBLK, 3], f32, kind="ExternalInput")

    w0p = nc.dram_tensor("w0p", [BLK, 8, NS], f16, kind="ExternalInput")
    w1p = nc.dram_tensor("w1p", [BLK, 2, NS], f16, kind="ExternalInput")
    w2p = nc.dram_tensor("w2p", [BLK, NS], f16, kind="ExternalInput")
    dfw1p = nc.dram_tensor("dfw1p", [BLK, 2, 1024], f16, kind="ExternalInput")
    dfw2p = nc.dram_tensor("dfw2p", [BLK, 8, NS], f16, kind="ExternalInput")
    miw1p = nc.dram_tensor("miw1p", [BLK, 4, 1024], f16, kind="ExternalInput")
    miw2p = nc.dram_tensor("miw2p", [BLK, 8, 1024], f16, kind="ExternalInput")
    mowp = nc.dram_tensor("mowp", [BLK, 8, BLK], f16, kind="ExternalInput")
    s0d = nc.dram_tensor("s0d", [BLK, 8 * BLK], f16, kind="ExternalInput")
    t0d = nc.dram_tensor("t0d", [BLK, BLK], f16, kind="ExternalInput")
    s1d = nc.dram_tensor("s1d", [BLK, 6 * BLK], f16, kind="ExternalInput")
    t1d = nc.dram_tensor("t1d", [BLK, 3 * BLK], f16, kind="ExternalInput")
    s2d = nc.dram_tensor("s2d", [BLK, 5 * BLK], f16, kind="ExternalInput")
    t2d = nc.dram_tensor("t2d", [BLK, 5 * BLK], f16, kind="ExternalInput")
    bdf1 = nc.dram_tensor("bdf1", [BLK, 8], f32, kind="ExternalInput")
    bdf2 = nc.dram_tensor("bdf2", [BLK, 4], f32, kind="ExternalInput")
    bmi1 = nc.dram_tensor("bmi1", [BLK, 8], f32, kind="ExternalInput")
    bmi2 = nc.dram_tensor("bmi2", [BLK, 8], f32, kind="ExternalInput")
    bmo = nc.dram_tensor("bmo", [1, 1], f32, kind="ExternalInput")
    cnd = nc.dram_tensor("cnd", [BLK, NB], f32, kind="ExternalInput")
    identd = nc.dram_tensor("identd", [BLK, BLK], f16, kind="ExternalInput")

    outd = nc.dram_tensor("out", [ntiles, 1, ET], f32, kind="ExternalOutput")

    TWO_PI = 2.0 * math.pi
    sin_bias = -math.pi if mode == "sim" else 0.0
    dscale_c = -EMBC if mode == "sim" else EMBC

    with tile.TileContext(nc) as tc:
        with (
            tc.tile_pool(name="const", bufs=1) as cp,
            tc.tile_pool(name="gat", bufs=3) as gp,
            tc.tile_pool(name="rad", bufs=3) as rp,
            tc.tile_pool(name="emb", bufs=2) as ep,
            tc.tile_pool(name="trx", bufs=3) as xp,
            tc.tile_pool(name="tpp", bufs=2) as tp,
            tc.tile_pool(name="mid", bufs=2) as mp,
            tc.tile_pool(name="row", bufs=1) as wp,
            tc.tile_pool(name="h1s", bufs=2) as hp1,
            tc.tile_pool(name="hs", bufs=2) as hp,
            tc.tile_pool(name="sml", bufs=2) as sp,
            tc.tile_pool(name="psA", bufs=2, space="PSUM") as pgA,
            tc.tile_pool(name="psB", bufs=2, space="PSUM") as pgB,
            tc.tile_pool(name="pac", bufs=3, space="PSUM") as pacc,
            tc.tile_pool(name="pst", bufs=1, space="PSUM") as pst,
        ):
            # ---------------- constants ----------------
            w0_t = cp.tile([BLK, 8, NS], f16)
            nc.sync.dma_start(w0_t[:], w0p[:])
            w1_t = cp.tile([BLK, 2, NS], f16)
            nc.sync.dma_start(w1_t[:], w1p[:])
            w2_t = cp.tile([BLK, NS], f16)
            nc.sync.dma_start(w2_t[:], w2p[:])
            dfw1_t = cp.tile([BLK, 2, 1024], f16)
            nc.sync.dma_start(dfw1_t[:], dfw1p[:])
            dfw2_t = cp.tile([BLK, 8, NS], f16)
            nc.sync.dma_start(dfw2_t[:], dfw2p[:])
            miw1_t = cp.tile([BLK, 4, 1024], f16)
            nc.sync.dma_start(miw1_t[:], miw1p[:])
            miw2_t = cp.tile([BLK, 8, 1024], f16)
            nc.sync.dma_start(miw2_t[:], miw2p[:])
            mow_t = cp.tile([BLK, 8, BLK], f16)
            nc.sync.dma_start(mow_t[:], mowp[:])
            s0_t = cp.tile([BLK, 8 * BLK], f16)
            nc.sync.dma_start(s0_t[:], s0d[:])
            t0_t = cp.tile([BLK, BLK], f16)
            nc.sync.dma_start(t0_t[:], t0d[:])
            s1_t = cp.tile([BLK, 6 * BLK], f16)
            nc.sync.dma_start(s1_t[:], s1d[:])
            t1_t = cp.tile([BLK, 3 * BLK], f16)
            nc.sync.dma_start(t1_t[:], t1d[:])
            s2_t = cp.tile([BLK, 5 * BLK], f16)
            nc.sync.dma_start(s2_t[:], s2d[:])
            t2_t = cp.tile([BLK, 5 * BLK], f16)
            nc.sync.dma_start(t2_t[:], t2d[:])
            bdf1_t = cp.tile([BLK, 8], f32)
            nc.sync.dma_start(bdf1_t[:], bdf1[:])
            bdf2_t = cp.tile([BLK, 4], f32)
            nc.sync.dma_start(bdf2_t[:], bdf2[:])
            bmi1_t = cp.tile([BLK, 8], f32)
            nc.sync.dma_start(bmi1_t[:], bmi1[:])
            bmi2_t = cp.tile([BLK, 8], f32)
            nc.sync.dma_start(bmi2_t[:], bmi2[:])
            bmo_t = cp.tile([1, 1], f32)
            nc.sync.dma_start(bmo_t[:], bmo[:])
            cn_t = cp.tile([BLK, NB], f32)
            nc.sync.dma_start(cn_t[:], cnd[:])
            ones128_t = cp.tile([BLK, BLK], f16)
            nc.gpsimd.memset(ones128_t[:], 1.0)
            id_t = cp.tile([BLK, BLK], f16)
            nc.sync.dma_start(id_t[:], identd[:])
            magic_t = cp.tile([BLK, NBLK], i32)
            nc.gpsimd.memset(magic_t[:], MAGIC)
            sinb_t = cp.tile([BLK, 1], f32)
            nc.gpsimd.memset(sinb_t[:], sin_bias)

            def silu_to(dst, ps, bias_ap):
                if mode == "sim":
                    sg = sp.tile([BLK, ET], f16, tag="sg")
                    nc.scalar.activation(sg[:], ps, AF.Sigmoid, bias=bias_ap, scale=1.0)
                    pre = sp.tile([BLK, ET], f16, tag="pre")
                    nc.vector.tensor_scalar(out=pre[:], in0=ps, scalar1=bias_ap,
                                            scalar2=None, op0=OP.add)
                    nc.vector.tensor_tensor(out=dst, in0=sg[:], in1=pre[:], op=OP.mult)
                else:
                    nc.scalar.activation(dst, ps, AF.Silu, bias=bias_ap, scale=1.0)

            def front1(t):
                # ============ index DMAs ============
                if stage < 0:
                    return None
                gs16 = gp.tile([BLK, ET // 16], i16, tag="gs16")
                gd16 = gp.tile([BLK, ET // 16], i16, tag="gd16")
                nc.sync.dma_start(gs16[:], g16s[t])
                nc.sync.dma_start(gd16[:], g16d[t])
                pci = gp.tile([BLK, 8], i32, tag="pci")
                nc.sync.dma_start(pci[:], pcidx[t])
                shf = gp.tile([BLK, NBLK, 3], f32, tag="shf")
                nc.sync.dma_start(shf[:], shiftd[t])

                # ============ gathers ============
                xsT = xp.tile([BLK, 1, ET], f16, tag="xsT")
                nc.gpsimd.dma_gather(
                    out_ap=xsT[:], in_ap=nodesF[:], idxs_ap=gs16[:],
                    num_idxs=ET, num_idxs_reg=ET, elem_size=BLK, transpose=True)
                xdT = xp.tile([BLK, 1, ET], f16, tag="xdT")
                nc.gpsimd.dma_gather(
                    out_ap=xdT[:], in_ap=nodesF[:], idxs_ap=gd16[:],
                    num_idxs=ET, num_idxs_reg=ET, elem_size=BLK, transpose=True)
                gsc = gp.tile([BLK, NBLK, 16], f32, tag="gsc")
                gdc = gp.tile([BLK, NBLK, 16], f32, tag="gdc")
                for b in range(NBLK):
                    nc.gpsimd.indirect_dma_start(
                        out=gsc[:, b, :], out_offset=None, in_=posC[:],
                        in_offset=bass.IndirectOffsetOnAxis(ap=pci[:, b:b + 1], axis=0))
                    nc.gpsimd.indirect_dma_start(
                        out=gdc[:, b, :], out_offset=None, in_=posC[:],
                        in_offset=bass.IndirectOffsetOnAxis(ap=pci[:, 4 + b:5 + b], axis=0))

                if stage < 1:
                    ot0 = sp.tile([1, ET], f32, tag="ot")
                    nc.vector.tensor_copy(ot0[:], xsT[0:1, 0, :])
                    nc.sync.dma_start(outd[t], ot0[:])
                    return None
                # ============ radial (f32) ============
                prod = rp.tile([BLK, NBLK, 3, 3], f32, tag="prod")
                nc.vector.tensor_tensor(
                    out=prod[:],
                    in0=gsc[:, :, 4:13].rearrange("p b (i j) -> p b j i", i=3, j=3),
                    in1=shf[:].unsqueeze(2).to_broadcast([BLK, NBLK, 3, 3]),
                    op=OP.mult)
                tvec = rp.tile([BLK, NBLK, 3], f32, tag="tvec")
                nc.vector.tensor_reduce(out=tvec[:], in_=prod[:], axis=AX.X, op=OP.add)
                rv = rp.tile([BLK, NBLK, 3], f32, tag="rv")
                nc.vector.tensor_tensor(out=rv[:], in0=gdc[:, :, 0:3],
                                        in1=gsc[:, :, 0:3], op=OP.subtract)
                nc.vector.tensor_tensor(out=rv[:], in0=rv[:], in1=tvec[:], op=OP.add)
                sq = rp.tile([BLK, NBLK, 3], f32, tag="sq")
                nc.vector.tensor_tensor(out=sq[:], in0=rv[:], in1=rv[:], op=OP.mult)
                d2 = rp.tile([BLK, NBLK], f32, tag="d2")
                nc.vector.tensor_reduce(out=d2[:], in_=sq[:], axis=AX.X, op=OP.add)
                nc.vector.tensor_scalar(out=d2[:], in0=d2[:], scalar1=1e-24,
                                        scalar2=None, op0=OP.max)
                # Newton rsqrt (2 iters)
                sh = rp.tile([BLK, NBLK], i32, tag="sh")
                nc.vector.tensor_scalar(out=sh[:], in0=d2[:].bitcast(i32), scalar1=1,
                                        scalar2=None, op0=OP.arith_shift_right)
                yi = rp.tile([BLK, NBLK], i32, tag="yi")
                nc.vector.tensor_tensor(out=yi[:], in0=magic_t[:], in1=sh[:],
                                        op=OP.subtract)
                y = yi[:].bitcast(f32)
                d2h = rp.tile([BLK, NBLK], f32, tag="d2h")
                nc.vector.tensor_scalar(out=d2h[:], in0=d2[:], scalar1=0.5,
                                        scalar2=None, op0=OP.mult)
                tmp = rp.tile([BLK, NBLK], f32, tag="tmp")
                for _ in range(2):
                    nc.vector.tensor_tensor(out=tmp[:], in0=y, in1=y, op=OP.mult)
                    nc.vector.tensor_tensor(out=tmp[:], in0=tmp[:], in1=d2h[:], op=OP.mult)
                    nc.vector.tensor_scalar(out=tmp[:], in0=tmp[:], scalar1=-1.0,
                                            scalar2=1.5, op0=OP.mult, op1=OP.add)
                    nc.vector.tensor_tensor(out=yi[:].bitcast(f32), in0=y, in1=tmp[:],
                                            op=OP.mult)
                dist = rp.tile([BLK, NBLK], f32, tag="dist")
                nc.vector.tensor_tensor(out=dist[:], in0=d2[:], in1=y, op=OP.mult)
                nc.vector.tensor_scalar(out=dist[:], in0=dist[:], scalar1=1e-6,
                                        scalar2=None, op0=OP.add)
                # r = 1/(dist+1e-6), one NR step from seed y
                nc.vector.tensor_tensor(out=tmp[:], in0=dist[:], in1=y, op=OP.mult)
                nc.vector.tensor_scalar(out=tmp[:], in0=tmp[:], scalar1=-1.0,
                                        scalar2=2.0, op0=OP.mult, op1=OP.add)
                r_ = rp.tile([BLK, NBLK], f32, tag="r_")
                nc.vector.tensor_tensor(out=r_[:], in0=y, in1=tmp[:], op=OP.mult)
                dsc = rp.tile([BLK, NBLK], f32, tag="dsc")
                nc.vector.tensor_scalar(out=dsc[:], in0=dist[:], scalar1=dscale_c,
                                        scalar2=None, op0=OP.mult)

                # ============ embedding (edge-major) ============
                u = ep.tile([BLK, NBLK, NB], f32, tag="u")
                nc.vector.tensor_tensor(
                    out=u[:], in0=r_[:].unsqueeze(2).to_broadcast([BLK, NBLK, NB]),
                    in1=cn_t[:].unsqueeze(1).to_broadcast([BLK, NBLK, NB]), op=OP.mult)
                icv = ep.tile([BLK, NBLK, NB], i16, tag="icv")
                nc.vector.tensor_copy(icv[:], u[:])
                nc.vector.tensor_tensor(out=u[:], in0=u[:], in1=icv[:], op=OP.subtract)
                sinv = ep.tile([BLK, NBLK * NB], f16, tag="sinv")
                sinv_v = sinv[:].rearrange("p (b n) -> p b n", b=NBLK)
                nc.scalar.activation(sinv_v, u[:], AF.Sin, bias=sinb_t[:, 0:1],
                                     scale=TWO_PI)
                for b in range(NBLK):
                    nc.scalar.activation(sinv_v[:, b, :], sinv_v[:, b, :], AF.Copy,
                                         scale=dsc[:, b:b + 1])

                if stage < 2:
                    ot0 = sp.tile([1, ET], f32, tag="ot")
                    nc.vector.tensor_copy(ot0[:], sinv[0:1, 0:ET])
                    nc.sync.dma_start(outd[t], ot0[:])
                    return None
                if stage < 3:
                    ot0 = sp.tile([1, ET], f32, tag="ot")
                    nc.vector.tensor_copy(ot0[:], embT[0:1, 0, :])
                    nc.sync.dma_start(outd[t], ot0[:])
                    return None
                # ============ TP replication + products ============
                prod16 = tp.tile([BLK, 11, ET], f16, tag="prod16")
                psB0 = pgB.tile([BLK, NS], f32, space="PSUM", tag="b")
                nc.tensor.matmul(psB0[:], t0_t[:], xdT[:, 0, :], start=True, stop=True)
                xd0t = tp.tile([BLK, ET], f16, tag="xd0t")
                nc.scalar.copy(xd0t[:], psB0[:])
                for c in range(8):
                    psA = pgA.tile([BLK, NS], f32, space="PSUM", tag="a")
                    nc.tensor.matmul(psA[:], s0_t[:, c * BLK:(c + 1) * BLK],
                                     xsT[:, 0, :], start=True, stop=True)
                    nc.vector.tensor_tensor(out=prod16[:, c, :], in0=psA[:],
                                            in1=xd0t[:], op=OP.mult)
                for i in range(3):
                    psB = pgB.tile([BLK, NS], f32, space="PSUM", tag="b")
                    nc.tensor.matmul(psB[:], t1_t[:, i * BLK:(i + 1) * BLK],
                                     xdT[:, 0, :], start=True, stop=True)
                    bsb = tp.tile([BLK, ET], f16, tag="bsb")
                    nc.scalar.copy(bsb[:], psB[:])
                    for c in range(2):
                        psA = pgA.tile([BLK, NS], f32, space="PSUM", tag="a")
                        nc.tensor.matmul(psA[:], s1_t[:, (c * 3 + i) * BLK:(c * 3 + i + 1) * BLK],
                                         xsT[:, 0, :], start=True, stop=True)
                        if i == 0:
                            nc.vector.tensor_tensor(out=prod16[:, 8 + c, :], in0=psA[:],
                                                    in1=bsb[:], op=OP.mult)
                        else:
                            tmq = tp.tile([BLK, ET], f16, tag="tmq")
                            nc.vector.tensor_tensor(out=tmq[:], in0=psA[:], in1=bsb[:],
                                                    op=OP.mult)
                            nc.vector.tensor_tensor(out=prod16[:, 8 + c, :],
                                                    in0=prod16[:, 8 + c, :],
                                                    in1=tmq[:], op=OP.add)
                for i in range(5):
                    psB = pgB.tile([BLK, NS], f32, space="PSUM", tag="b")
                    nc.tensor.matmul(psB[:], t2_t[:, i * BLK:(i + 1) * BLK],
                                     xdT[:, 0, :], start=True, stop=True)
                    bsb2 = tp.tile([BLK, ET], f16, tag="bsb2")
                    nc.scalar.copy(bsb2[:], psB[:])
                    psA = pgA.tile([BLK, NS], f32, space="PSUM", tag="a")
                    nc.tensor.matmul(psA[:], s2_t[:, i * BLK:(i + 1) * BLK],
                                     xsT[:, 0, :], start=True, stop=True)
                    if i == 0:
                        nc.vector.tensor_tensor(out=prod16[:, 10, :], in0=psA[:],
                                                in1=bsb2[:], op=OP.mult)
                    else:
                        tmq2 = tp.tile([BLK, ET], f16, tag="tmq")
                        nc.vector.tensor_tensor(out=tmq2[:], in0=psA[:],
                                                in1=bsb2[:], op=OP.mult)
                        nc.vector.tensor_tensor(out=prod16[:, 10, :],
                                                in0=prod16[:, 10, :],
                                                in1=tmq2[:], op=OP.add)

                if stage < 4:
                    ot0 = sp.tile([1, ET], f32, tag="ot")
                    nc.vector.tensor_copy(ot0[:], prod16[0:1, 0, :])
                    nc.sync.dma_start(outd[t], ot0[:])
                    return None
                return dict(prod16=prod16, sinv=sinv)

            def front2(t, s1):
                prod16 = s1["prod16"]
                sinv = s1["sinv"]
                # ===== mix einsum (c-order in m passes) + LN mu stats =====
                mix_sb = mp.tile([BLK, 4, NS], f16, tag="mix_sb")
                sqf = mp.tile([BLK, 4, NS], f16, tag="sqf")
                mu_ps = pst.tile([BLK, NS], f32, space="PSUM", tag="st")

                def mix_chunk(acc, c, m, start):
                    if c < 8:
                        nc.tensor.matmul(acc[:], w0_t[:, c, m * BLK:(m + 1) * BLK],
                                         prod16[:, c, :], start=start, stop=False)
                    elif c < 10:
                        nc.tensor.matmul(acc[:], w1_t[:, c - 8, m * BLK:(m + 1) * BLK],
                                         prod16[:, c, :], start=start, stop=False)
                    else:
                        nc.tensor.matmul(acc[:], w2_t[:, m * BLK:(m + 1) * BLK],
                                         prod16[:, 10, :], start=start, stop=True)

                for m in range(4):
                    acc = pacc.tile([BLK, NS], f32, space="PSUM", tag="a")
                    for c in range(11):
                        mix_chunk(acc, c, m, start=(c == 0))
                    nc.scalar.copy(mix_sb[:, m, :], acc[:])
                    nc.vector.tensor_tensor(out=sqf[:, m, :], in0=mix_sb[:, m, :],
                                            in1=mix_sb[:, m, :], op=OP.mult)
                    nc.tensor.matmul(mu_ps[:], ones128_t[:], mix_sb[:, m, :],
                                     start=(m == 0), stop=(m == 3))

                if stage < 5:
                    ot0 = sp.tile([1, ET], f32, tag="ot")
                    nc.vector.tensor_copy(ot0[:], mix_sb[0:1, 0, :])
                    nc.sync.dma_start(outd[t], ot0[:])
                    return None

                # mu consumed early so the stats bank can be reused for sq
                rmu = wp.tile([BLK, NS], f32, tag="rmu")
                nc.vector.tensor_scalar(out=rmu[:], in0=mu_ps[:], scalar1=1.0 / NS,
                                        scalar2=None, op0=OP.mult)
                s2_ps = pst.tile([BLK, NS], f32, space="PSUM", tag="st")

                # embT via PE transposes (fp16, 1c/row)
                embT = xp.tile([BLK, 2, ET], f16, tag="embT")
                sinb = sinv[:].rearrange("p (b n) -> p b n", b=NBLK)
                for b in range(NBLK):
                    for k in range(2):
                        pt = pgA.tile([BLK, NS], f32, space="PSUM", tag="a")
                        ptv = pt[:].bitcast(f16)[:, 0:BLK]
                        nc.tensor.transpose(ptv,
                                            sinb[:, b, k * BLK:(k + 1) * BLK],
                                            id_t[:])
                        nc.scalar.copy(embT[:, k, b * BLK:(b + 1) * BLK], ptv)

                # ============ df MLP layer 1 (+ sq stats interleaved) ======
                h1c = hp1.tile([BLK, 8, ET], f16, tag="h1c")
                for m in range(8):
                    acc = pacc.tile([BLK, NS], f32, space="PSUM", tag="a")
                    for k in range(2):
                        nc.tensor.matmul(acc[:], dfw1_t[:, k, m * BLK:(m + 1) * BLK],
                                         embT[:, k, :], start=(k == 0), stop=(k == 1))
                    silu_to(h1c[:, m, :], acc[:], bdf1_t[:, m:m + 1])
                    if m < 4:
                        nc.tensor.matmul(s2_ps[:], ones128_t[:], sqf[:, m, :],
                                         start=(m == 0), stop=(m == 3))
                # ============ df MLP layer 2 ============
                dff = mp.tile([BLK, 4, NS], f16, tag="dff")
                for m in range(4):
                    acc = pacc.tile([BLK, NS], f32, space="PSUM", tag="a")
                    for k in range(8):
                        nc.tensor.matmul(acc[:], dfw2_t[:, k, m * BLK:(m + 1) * BLK],
                                         h1c[:, k, :], start=(k == 0), stop=(k == 7))
                    nc.scalar.activation(dff[:, m, :], acc[:], AF.Identity,
                                         bias=bdf2_t[:, m:m + 1], scale=1.0)

                # ============ LN chain, full [128, NS] width (f32) ========
                rt = wp.tile([BLK, NS], f32, tag="rt")
                nc.vector.tensor_tensor(out=rt[:], in0=rmu[:], in1=rmu[:], op=OP.mult)
                rvv = wp.tile([BLK, NS], f32, tag="rvv")
                nc.vector.scalar_tensor_tensor(out=rvv[:], in0=s2_ps[:],
                                               scalar=1.0 / NS, in1=rt[:],
                                               op0=OP.mult, op1=OP.subtract)
                nc.vector.tensor_scalar(out=rvv[:], in0=rvv[:], scalar1=1e-5,
                                        scalar2=None, op0=OP.add)
                shv = wp.tile([BLK, NS], i32, tag="shv")
                nc.vector.tensor_scalar(out=shv[:], in0=rvv[:].bitcast(i32), scalar1=1,
                                        scalar2=None, op0=OP.arith_shift_right)
                # yi = MAGIC - sh  ==  (sh - MAGIC) * -1
                nc.vector.tensor_scalar(out=shv[:], in0=shv[:], scalar1=MAGIC,
                                        scalar2=-1, op0=OP.subtract, op1=OP.mult)
                ry = shv[:].bitcast(f32)
                nc.vector.tensor_tensor(out=rt[:], in0=ry, in1=ry, op=OP.mult)
                nc.vector.scalar_tensor_tensor(out=rt[:], in0=rt[:],
                                               scalar=-0.5, in1=rvv[:],
                                               op0=OP.mult, op1=OP.mult)
                Asb = mp.tile([BLK, NS], f16, tag="Asb")
                nc.vector.scalar_tensor_tensor(out=Asb[:], in0=rt[:], scalar=1.5,
                                               in1=ry, op0=OP.add, op1=OP.mult)
                Bsb = mp.tile([BLK, NS], f16, tag="Bsb")
                nc.vector.tensor_tensor(out=Bsb[:], in0=rmu[:], in1=Asb[:],
                                        op=OP.mult)

                # ============ reg = (mix - mu)*rstd*g (*) df ============
                for k in range(4):
                    nc.vector.tensor_tensor(out=sqf[:, k, :], in0=mix_sb[:, k, :],
                                            in1=Asb[:], op=OP.mult)
                    nc.vector.tensor_tensor(out=sqf[:, k, :], in0=sqf[:, k, :],
                                            in1=Bsb[:], op=OP.subtract)
                    nc.vector.tensor_tensor(out=dff[:, k, :], in0=sqf[:, k, :],
                                            in1=dff[:, k, :], op=OP.mult)
                return dict(reg=dff)

            def back(t, fs):
                reg = fs["reg"]

                # ============ mix MLP ============
                h = hp.tile([BLK, 8, ET], f16, tag="h")
                for m in range(8):
                    acc = pacc.tile([BLK, NS], f32, space="PSUM", tag="a")
                    for k in range(4):
                        nc.tensor.matmul(acc[:], miw1_t[:, k, m * BLK:(m + 1) * BLK],
                                         reg[:, k, :], start=(k == 0), stop=(k == 3))
                    silu_to(h[:, m, :], acc[:], bmi1_t[:, m:m + 1])
                po = pgB.tile([BLK, NS], f32, space="PSUM", tag="b")
                for m in range(8):
                    acc = pacc.tile([BLK, NS], f32, space="PSUM", tag="a")
                    for k in range(8):
                        nc.tensor.matmul(acc[:], miw2_t[:, k, m * BLK:(m + 1) * BLK],
                                         h[:, k, :], start=(k == 0), stop=(k == 7))
                    h2m = sp.tile([BLK, ET], f16, tag="h2m")
                    silu_to(h2m[:], acc[:], bmi2_t[:, m:m + 1])
                    nc.tensor.matmul(po[:], mow_t[:, m, :], h2m[:],
                                     start=(m == 0), stop=(m == 7))
                ot = sp.tile([1, ET], f32, tag="ot")
                nc.scalar.activation(ot[:], po[0:1, :], AF.Identity, bias=bmo_t[:, 0:1],
                                     scale=1.0)
                nc.sync.dma_start(outd[t], ot[:])

            # 3-slot software pipeline: per iteration the PE FIFO gets
            # [FRONT1(t+1): replication]  [BACK(t): mi1/mi2/mo ~104 MMs]
            # [FRONT2(t+1): mix/stats/df].  BACK(t)'s dense matmul block
            # hides FRONT1(t+1)'s serial DVE product phase (~14us), and
            # FRONT2's matmuls hide BACK's silu/LN chains.
            pend = None
            for t in [tt for _ in range(reps) for tt in range(ntiles)]:
                s1 = front1(t)
                if pend is not None:
                    back(*pend)
                    pend = None
                if s1 is None:
                    continue
                fs = front2(t, s1)
                if fs is None:
                    continue
                pend = (t, fs)
            if pend is not None:
                back(*pend)

    nc.finalize()
    return nc


def _wrap16(v):
    """dma_gather index layout: idx i at [i % 16, i // 16], replicated to
    128 partitions (8 gpsimd cores x 16)."""
    n = v.shape[-1]
    w = np.zeros(v.shape[:-1] + (128, n // 16), np.int16)
    r = v.reshape(v.shape[:-1] + (n // 16, 16))
    for rep in range(8):
        w[..., 16 * rep:16 * (rep + 1), :] = np.swapaxes(r, -1, -2)
    return w


def _host_prep(inputs):
    """Shared (replicated) host-side tensors."""
    f = np.float32
    nodes = np.asarray(inputs["nodes"], f)
    pos = np.asarray(inputs["pos"], f)
    cell = np.asarray(inputs["cell"], f)
    W0 = np.asarray(inputs["W0"], f)
    W1 = np.asarray(inputs["W1"], f)
    W2 = np.asarray(inputs["W2"], f)
    ln_g = np.asarray(inputs["ln_g"], f)

    nodesF = np.zeros((N, BLK), np.float16)
    nodesF[:, :FEAT] = nodes.astype(np.float16)
    bv = np.asarray(inputs["batch_vec"]).astype(np.int64)
    posC = np.zeros((N, 16), f)
    posC[:, :3] = pos
    posC[:, 4:13] = cell.reshape(G, 9)[bv]

    sym = lambda W: 0.5 * (W + W.transpose(1, 0, 2))
    w0f = (sym(W0) / FAN).reshape(L0 * L0, NS)
    w1f = (sym(W1) / (FAN * math.sqrt(3.0))).reshape(L1 * L1, NS)
    w2f = (sym(W2) / (FAN * math.sqrt(5.0))).reshape(L2 * L2, NS)
    h16 = np.float16

    def chunk(w, nch):
        return np.ascontiguousarray(
            w.reshape(nch, BLK, w.shape[1]).transpose(1, 0, 2)).astype(h16)

    miw1 = ln_g[:, None] * np.asarray(inputs["mi_w1"], f)

    def colbias(b, nch):
        b = np.asarray(b, f).reshape(nch, BLK)
        return np.ascontiguousarray(b.T)

    O0, O1, O2 = 0, L0, L0 + 3 * L1
    s0 = np.zeros((BLK, 8 * BLK), h16)
    for c in range(8):
        for p in range(BLK):
            s0[O0 + c * 4 + p // 32, c * BLK + p] = 1.0
    t0 = np.zeros((BLK, BLK), h16)
    for p in range(BLK):
        t0[O0 + p % 32, p] = 1.0
    s1 = np.zeros((BLK, 6 * BLK), h16)
    for c in range(2):
        for i in range(3):
            for p in range(BLK):
                u = c * 8 + p // 16
                s1[O1 + u * 3 + i, (c * 3 + i) * BLK + p] = 1.0
    t1 = np.zeros((BLK, 3 * BLK), h16)
    for i in range(3):
        for p in range(BLK):
            t1[O1 + (p % 16) * 3 + i, i * BLK + p] = 1.0
    # l2 selection matrices, column-padded to 128 per component so FWL
    # stays on; cols 64..127 of each chunk are zero -> psA/psB partitions
    # 64..127 come out zero and the products/mix stay exact.
    s2 = np.zeros((BLK, 5 * BLK), h16)
    t2 = np.zeros((BLK, 5 * BLK), h16)
    for i in range(5):
        for p in range(64):
            s2[O2 + (p // 8) * 5 + i, i * BLK + p] = 1.0
            t2[O2 + (p % 8) * 5 + i, i * BLK + p] = 1.0
    w2full = np.zeros((BLK, NS), h16)
    w2full[:64] = w2f.astype(h16)
    mow = np.asarray(inputs["mo_w"], f).reshape(8, BLK)
    mow128 = np.zeros((BLK, 8, BLK), h16)
    for m in range(8):
        mow128[:, m, 0] = mow[m]
    cn = np.broadcast_to((np.arange(1, NB + 1, dtype=f) / (2.0 * CUT))[None, :],
                         (BLK, NB)).copy()
    return dict(
        nodesF=nodesF, posC=posC,
        w0p=chunk(w0f, 8), w1p=chunk(w1f, 2),
        w2p=w2full,
        dfw1p=chunk(np.asarray(inputs["df_w1"], f), 2),
        dfw2p=chunk(np.asarray(inputs["df_w2"], f), 8),
        miw1p=chunk(miw1, 4),
        miw2p=chunk(np.asarray(inputs["mi_w2"], f), 8),
        mowp=mow128,
        s0d=s0, t0d=t0, s1d=s1, t1d=t1, s2d=s2, t2d=t2,
        bdf1=colbias(inputs["df_b1"], 8), bdf2=colbias(inputs["df_b2"], 4),
        bmi1=colbias(inputs["mi_b1"], 8), bmi2=colbias(inputs["mi_b2"], 8),
        bmo=np.asarray(inputs["mo_b"], f).reshape(1, 1),
        cnd=cn,
        identd=np.eye(BLK, dtype=h16),
    )


def _edge_prep(inputs, core, ntiles):
    """Per-core edge tensors."""
    f = np.float32
    ec = ntiles * ET
    lo = core * EC
    ei = np.asarray(inputs["edge_index"])
    src = ei[0, lo:lo + ec].astype(np.int32)
    dst = ei[1, lo:lo + ec].astype(np.int32)
    shift = np.asarray(inputs["edge_shift"], f)[lo:lo + ec]

    def tile_idx(x):
        return np.ascontiguousarray(x.reshape(ntiles, NBLK, BLK).transpose(0, 2, 1))

    pcidx = np.concatenate([tile_idx(src), tile_idx(dst)], axis=2)

    return dict(
        g16s=_wrap16(src.reshape(ntiles, ET).astype(np.int16)),
        g16d=_wrap16(dst.reshape(ntiles, ET).astype(np.int16)),
        pcidx=pcidx,
        shiftd=np.ascontiguousarray(
            shift.reshape(ntiles, NBLK, BLK, 3).transpose(0, 2, 1, 3)),
    )


def _run(inputs, mode, ntiles, ncores):
    key = (mode, ntiles, 1)
    if key not in _cache:
        _cache[key] = _build(mode, ntiles)
    nc = _cache[key]
    shared = _host_prep(inputs)
    in_maps = []
    for c in range(ncores):
        m = dict(shared)
        m.update(_edge_prep(inputs, c, ntiles))
        in_maps.append(m)

    if mode == "sim":
        from concourse.bass_interp import CoreSim
        outs = []
        for c in range(ncores):
            sim = CoreSim(nc)
            for k, v in in_maps[c].items():
                sim.tensor(k)[:] = v
            sim.simulate()
            outs.append(np.array(sim.tensor("out")).reshape(-1))
        return np.concatenate(outs).reshape(-1, 1)

    from concourse.bass_utils import run_bass_kernel_spmd
    trace = os.environ.get("EXB_TRACE", "0") == "1"
    res = run_bass_kernel_spmd(nc, in_maps, list(range(ncores)), trace=trace)
    out = np.concatenate([res.results[c]["out"].reshape(-1) for c in range(ncores)])
    if trace:
        _run.last_exec_time_ns = res.exec_time_ns
    return out.reshape(-1, 1)


def kernel(**inputs) -> np.ndarray:
    return _run(inputs, os.environ.get("EXB_MODE", "hw"), EC // ET, NCORES).astype(np.float32)
